# revision 1
# baseline (speedup 1.0000x reference)
"""EvolveGCN (2-layer) Trainium2 Bass kernel, 8-way sharded.

Key algebraic reduction: the mat-GRU that evolves the GCN weights depends only
on the previous weights (never on data), and layer outputs at time t depend
only on inputs at time t.  Since the model returns h2[T-1] only, the entire
computation collapses to:

    W1* = matGRU^4(W1);  W2* = matGRU^4(W2)        (tiny 128x128 host math)
    h1  = rrelu(A3 @ (X3 @ W1*))
    out = rrelu(A3 @ (h1 @ W2*))

Sharding: output rows (nodes) are range-partitioned across the 8 cores.  Each
core builds its [6250,128] slice of the dense table (X@W / h1@W), an AllGather
replicates the fp16 table to every core, and per-core SWDGE dma_gather pulls
the per-edge messages.  The sparse scatter (segment-sum by row) is done as a
sequence of one-hot-times-val fp16 matmuls on the tensor engine: edges are
host-sorted into 32-row windows, padded to 128-edge chunks; each chunk's S
block [128 edges, 32 rows] carries val at (edge, row) so PSUM accumulates the
weighted segment sums directly.  dma_gather indices are int16, so edges are
split into two groups (table row < 32768 / >= 32768) gathered with different
table base offsets.  Both layers share the same adjacency (t=3), so the idx/S
structures are built once and used twice.
"""

import sys
import numpy as np

for _p in ("/opt/trn_rl_repo",):
    if _p not in sys.path:
        sys.path.insert(0, _p)

T, N, E, F = 4, 50000, 800000, 128
NC = 8
NPC = N // NC            # 6250 nodes per core
RTP = 6272               # padded rows per core (49 tiles of 128)
NT = RTP // 128          # 49 row tiles per core
WROWS = 64               # scatter window rows (matmul M)
NW = RTP // WROWS        # 196 windows per core
SPLIT = 32768            # int16 index limit
SLOPE = 11.0 / 48.0      # torch RReLU eval negative slope
SEGP = 2                 # row tiles per gather segment

# debug bisection flags
DBG_NO_GATHER = False
DBG_NO_SPMM_MM = False
DBG_ONE_LAYER = False
REPS = 1  # timing: chain the whole pipeline N times
PHASE = "all"  # all | table | gather | mm  (timing bisection)
SIM1 = False  # single-core, no-collective variant for TimelineSim


def _evolve(W0, gW, gU, gb, steps=T):
    def sig(x):
        return 1.0 / (1.0 + np.exp(-x))

    Q = W0.astype(np.float64)
    gW = gW.astype(np.float64)
    gU = gU.astype(np.float64)
    gb = gb.astype(np.float64)
    for _ in range(steps):
        z = sig(gW[0] @ Q + gU[0] @ Q + gb[0])
        r = sig(gW[1] @ Q + gU[1] @ Q + gb[1])
        h = np.tanh(gW[2] @ Q + gU[2] @ (r * Q) + gb[2])
        Q = (1.0 - z) * Q + z * h
    return Q.astype(np.float32)


def _prep_edges(row, col, val):
    """Host-side edge schedule. Returns per-core input arrays + shared chunk
    schedule (identical across cores, baked into the single SPMD program)."""
    tcol = (col // NPC) * RTP + (col % NPC)     # remapped table row
    corei = row // NPC
    rl = row % NPC
    win = rl // WROWS
    rr = rl % WROWS
    grp = (tcol >= SPLIT).astype(np.int64)

    # counts[core, grp, win]
    counts = np.zeros((NC, 2, NW), np.int64)
    np.add.at(counts, (corei, grp, win), 1)
    # chunks per (grp, win): max over cores so one schedule fits all
    CC = -(-counts // 128)      # ceildiv
    CC = CC.max(axis=0)         # [2, NW]
    # ensure every window has >= 1 chunk so its PSUM rows get written
    empty = (CC[0] + CC[1]) == 0
    CC[0][empty] = 1

    baseA = np.zeros(NW + 1, np.int64)
    baseA[1:] = np.cumsum(CC[0])
    baseB = np.zeros(NW + 1, np.int64)
    baseB[1:] = np.cumsum(CC[1])
    NCHA, NCHB = int(baseA[-1]), int(baseB[-1])
    NA, NB = NCHA * 128, NCHB * 128

    idxa = np.zeros((NC, 128, NA // 16), np.int16)
    idxb = np.zeros((NC, 128, NB // 16), np.int16)
    sa = np.zeros((NC, 128, NCHA * WROWS), np.float16)
    sb = np.zeros((NC, 128, NCHB * WROWS), np.float16)

    for i in range(NC):
        for g, (base, idxg, sg) in enumerate(
            ((baseA, idxa, sa), (baseB, idxb, sb))
        ):
            m = (corei == i) & (grp == g)
            ew, err = win[m], rr[m]
            etc = tcol[m] - g * SPLIT
            ev = val[m]
            # stable order by window; slot within window = running position
            order = np.argsort(ew, kind="stable")
            ew, err, etc, ev = ew[order], err[order], etc[order], ev[order]
            # slot index within each window
            winstart = np.searchsorted(ew, np.arange(NW))
            pos = np.arange(ew.size) - winstart[ew]
            slot = base[ew] * 128 + pos
            assert (pos < (base[ew + 1] - base[ew]) * 128).all()
            # gather idx array: edge e at [e%16, e//16]
            flat = np.zeros(base[-1] * 128, np.int16)
            flat[slot] = etc.astype(np.int16)
            idxg[i][:16] = flat.reshape(-1, 16).T
            idxg[i] = np.tile(idxg[i][:16], (8, 1))
            # S: [partition = slot%128, (chunk = slot//128)*WROWS + rr] = val
            sflat = sg[i].reshape(-1)
            sidx = (slot % 128) * (base[-1] * WROWS) + (slot // 128) * WROWS + err
            sflat[sidx] = ev.astype(np.float16)

    return CC, baseA, baseB, idxa, idxb, sa, sb


def _build_program(CC, baseA, baseB, NCHA, NCHB):
    import concourse.bass as bass
    import concourse.tile as tile
    from concourse import bacc, mybir
    from concourse.masks import make_identity
    from contextlib import ExitStack

    F32, F16, I16 = mybir.dt.float32, mybir.dt.float16, mybir.dt.int16
    NA, NB = NCHA * 128, NCHB * 128

    nc = bacc.Bacc(
        "TRN2", target_bir_lowering=False, debug=False,
        num_devices=(1 if SIM1 else NC),
    )
    xs_d = nc.dram_tensor("xs", [RTP, F], F32, kind="ExternalInput")
    w1_d = nc.dram_tensor("w1", [F, F], F32, kind="ExternalInput")
    w2_d = nc.dram_tensor("w2", [F, F], F32, kind="ExternalInput")
    idxa_d = nc.dram_tensor("idxa", [128, NA // 16], I16, kind="ExternalInput")
    idxb_d = nc.dram_tensor("idxb", [128, NB // 16], I16, kind="ExternalInput")
    sa_d = nc.dram_tensor("sa", [128, NCHA * WROWS], F16, kind="ExternalInput")
    sb_d = nc.dram_tensor("sb", [128, NCHB * WROWS], F16, kind="ExternalInput")
    out_d = nc.dram_tensor("out", [RTP, F], F32, kind="ExternalOutput")

    # gather segments: SEGP row tiles each
    segs = []
    for p0 in range(0, NT, SEGP):
        p1 = min(p0 + SEGP, NT)
        w0, w1 = p0 * (128 // WROWS), p1 * (128 // WROWS)
        segs.append((p0, p1, w0, w1))
    max_cha = max(int(baseA[w1] - baseA[w0]) for _, _, w0, w1 in segs)
    max_chb = max(int(baseB[w1] - baseB[w0]) for _, _, w0, w1 in segs)

    with tile.TileContext(nc) as tc, ExitStack() as ctx:
        const = ctx.enter_context(tc.tile_pool(name="const", bufs=1))
        xin = ctx.enter_context(tc.tile_pool(name="xin", bufs=8))
        tps = ctx.enter_context(tc.tile_pool(name="tps", bufs=2, space="PSUM"))
        xtp = ctx.enter_context(tc.tile_pool(name="xtp", bufs=2))
        tsh = ctx.enter_context(tc.tile_pool(name="tsh", bufs=8))
        accp = ctx.enter_context(tc.tile_pool(name="accp", bufs=6, space="PSUM"))
        msgp = ctx.enter_context(tc.tile_pool(name="msgp", bufs=3))
        rrp = ctx.enter_context(tc.tile_pool(name="rrp", bufs=4))
        big = ctx.enter_context(tc.tile_pool(name="big", bufs=1))
        dram = ctx.enter_context(tc.tile_pool(name="dram", bufs=1, space="DRAM"))

        ident = const.tile([128, 128], F32)
        make_identity(nc, ident[:])
        w1_sb = const.tile([F, F], F32)
        nc.sync.dma_start(w1_sb[:], w1_d[:, :])
        w2_sb = const.tile([F, F], F32)
        nc.sync.dma_start(w2_sb[:], w2_d[:, :])
        idxa_sb = big.tile([128, NA // 16], I16)
        nc.sync.dma_start(idxa_sb[:], idxa_d[:, :])
        idxb_sb = big.tile([128, NB // 16], I16)
        nc.sync.dma_start(idxb_sb[:], idxb_d[:, :])
        sa_sb = big.tile([128, NCHA * WROWS], F16)
        nc.sync.dma_start(sa_sb[:], sa_d[:, :])
        sb_sb = big.tile([128, NCHB * WROWS], F16)
        nc.sync.dma_start(sb_sb[:], sb_d[:, :])
        h1_sb = big.tile([128, NT * 128], F32)

        def build_table(w_sb, shard, table, from_dram):
            for t in range(NT):
                if from_dram:
                    xt_in = xin.tile([128, 128], F32, tag="xin")
                    nc.sync.dma_start(xt_in[:], xs_d[t * 128 : (t + 1) * 128, :])
                    src = xt_in[:]
                else:
                    src = h1_sb[:, t * 128 : (t + 1) * 128]
                tp = tps.tile([128, 128], F32, tag="tp")
                nc.tensor.transpose(tp[:], src, ident[:])
                xts = xtp.tile([128, 128], F32, tag="xts")
                nc.vector.tensor_copy(xts[:], tp[:])
                mp = tps.tile([128, 128], F32, tag="tp")
                nc.tensor.matmul(
                    out=mp[:], lhsT=xts[:], rhs=w_sb[:], start=True, stop=True
                )
                sh = tsh.tile([128, 128], F16, tag="sh")
                nc.scalar.activation(
                    sh[:], mp[:], mybir.ActivationFunctionType.Copy
                )
                nc.sync.dma_start(shard[t * 128 : (t + 1) * 128, :], sh[:])
            if SIM1:
                for r in range(NC):
                    nc.sync.dma_start(table[r * RTP : (r + 1) * RTP, :], shard[:])
            else:
                nc.gpsimd.collective_compute(
                    "AllGather",
                    mybir.AluOpType.bypass,
                    replica_groups=[list(range(NC))],
                    ins=[shard.opt()],
                    outs=[table.opt()],
                )

        def spmm(table, emit):
            for si, (p0, p1, w0, w1) in enumerate(segs):
                ca0, ca1 = int(baseA[w0]), int(baseA[w1])
                cb0, cb1 = int(baseB[w0]), int(baseB[w1])
                na, nb = (ca1 - ca0) * 128, (cb1 - cb0) * 128
                msga = msgp.tile([128, max_cha, 128], F16, tag="msga")
                msgb = msgp.tile([128, max_chb, 128], F16, tag="msgb")
                if DBG_NO_GATHER or PHASE == "mm":
                    nc.vector.memset(msga[:], 0)
                    nc.vector.memset(msgb[:], 0)
                skip_g = DBG_NO_GATHER or PHASE == "mm"
                if na and not skip_g:
                    nc.gpsimd.dma_gather(
                        out_ap=msga[:, : ca1 - ca0, :],
                        in_ap=table[:SPLIT, :],
                        idxs_ap=idxa_sb[:, ca0 * 8 : ca1 * 8],
                        num_idxs=na,
                        num_idxs_reg=na,
                        elem_size=F,
                        single_packet=False,
                    )
                if nb and not skip_g:
                    nc.gpsimd.dma_gather(
                        out_ap=msgb[:, : cb1 - cb0, :],
                        in_ap=table[SPLIT:, :],
                        idxs_ap=idxb_sb[:, cb0 * 8 : cb1 * 8],
                        num_idxs=nb,
                        num_idxs_reg=nb,
                        elem_size=F,
                        single_packet=False,
                    )
                if PHASE == "gather":
                    continue
                WQ = 128 // WROWS
                for pt in range(p0, p1):
                    acc = accp.tile([128, 128], mybir.dt.float32, tag="acc")
                    if DBG_NO_SPMM_MM:
                        nc.vector.tensor_scalar_mul(acc[:], msga[:, 0, :], 0.0)
                        emit(pt, acc)
                        continue
                    for q in range(WQ):
                        w = pt * WQ + q
                        nw_ch = int(
                            baseA[w + 1] - baseA[w] + baseB[w + 1] - baseB[w]
                        )
                        k = 0
                        for gc in range(int(baseA[w]), int(baseA[w + 1])):
                            nc.tensor.matmul(
                                out=acc[WROWS * q : WROWS * (q + 1), :],
                                lhsT=sa_sb[:, gc * WROWS : (gc + 1) * WROWS],
                                rhs=msga[:, gc - ca0, :],
                                start=(k == 0),
                                stop=(k == nw_ch - 1),
                                tile_position=(0, WROWS * q),
                            )
                            k += 1
                        for gc in range(int(baseB[w]), int(baseB[w + 1])):
                            nc.tensor.matmul(
                                out=acc[WROWS * q : WROWS * (q + 1), :],
                                lhsT=sb_sb[:, gc * WROWS : (gc + 1) * WROWS],
                                rhs=msgb[:, gc - cb0, :],
                                start=(k == 0),
                                stop=(k == nw_ch - 1),
                                tile_position=(0, WROWS * q),
                            )
                            k += 1
                    emit(pt, acc)

        if PHASE in ("gather", "mm"):
            # tables built once; spmm phase repeated
            shard1 = dram.tile([RTP, F], F16)
            table1 = dram.tile([NC * RTP, F], F16, addr_space="Shared")
            build_table(w1_sb, shard1, table1, from_dram=True)

            def emitp(pt, acc):
                tmp = rrp.tile([128, 128], F32, tag="rtmp")
                nc.vector.tensor_scalar_mul(tmp[:], acc[:], SLOPE)
                res = rrp.tile([128, 128], F32, tag="res")
                nc.vector.tensor_tensor(
                    out=res[:], in0=tmp[:], in1=acc[:], op=mybir.AluOpType.max
                )
                nc.sync.dma_start(out_d[pt * 128 : (pt + 1) * 128, :], res[:])

            for _rep in range(REPS):
                spmm(table1, emitp)

        # ---- layer 1
        for _rep in range(REPS if PHASE not in ("gather", "mm") else 0):
            shard1 = dram.tile([RTP, F], F16, name=f"shard1_{_rep}")
            shard2 = dram.tile([RTP, F], F16, name=f"shard2_{_rep}")
            _aspace = "Local" if SIM1 else "Shared"
            table1 = dram.tile([NC * RTP, F], F16, addr_space=_aspace, name=f"table1_{_rep}")
            table2 = dram.tile([NC * RTP, F], F16, addr_space=_aspace, name=f"table2_{_rep}")
            build_table(w1_sb, shard1, table1, from_dram=True)
            if PHASE == "table":
                continue

            def emit1(pt, acc):
                tmp = rrp.tile([128, 128], F32, tag="rtmp")
                nc.vector.tensor_scalar_mul(tmp[:], acc[:], SLOPE)
                nc.vector.tensor_tensor(
                    out=h1_sb[:, pt * 128 : (pt + 1) * 128],
                    in0=tmp[:],
                    in1=acc[:],
                    op=mybir.AluOpType.max,
                )

            spmm(table1, emit1)

            # ---- layer 2
            build_table(w2_sb, shard2, table2, from_dram=False)

            def emit2(pt, acc):
                tmp = rrp.tile([128, 128], F32, tag="rtmp")
                nc.vector.tensor_scalar_mul(tmp[:], acc[:], SLOPE)
                res = rrp.tile([128, 128], F32, tag="res")
                nc.vector.tensor_tensor(
                    out=res[:], in0=tmp[:], in1=acc[:], op=mybir.AluOpType.max
                )
                nc.sync.dma_start(out_d[pt * 128 : (pt + 1) * 128, :], res[:])

            spmm(table2, emit2)

    nc.compile()
    return nc


def kernel(
    features,
    adj_row,
    adj_col,
    adj_val,
    W1,
    g1_W,
    g1_U,
    g1_b,
    W2,
    g2_W,
    g2_U,
    g2_b,
    _run_kwargs=None,
):
    from concourse.bass_utils import run_bass_kernel_spmd

    X = np.asarray(features[T - 1], dtype=np.float32)
    row = np.asarray(adj_row[T - 1], dtype=np.int64)
    col = np.asarray(adj_col[T - 1], dtype=np.int64)
    val = np.asarray(adj_val[T - 1], dtype=np.float32)

    W1f = _evolve(np.asarray(W1), np.asarray(g1_W), np.asarray(g1_U), np.asarray(g1_b))
    W2f = _evolve(np.asarray(W2), np.asarray(g2_W), np.asarray(g2_U), np.asarray(g2_b))

    CC, baseA, baseB, idxa, idxb, sa, sb = _prep_edges(row, col, val)
    NCHA, NCHB = int(baseA[-1]), int(baseB[-1])

    nc = _build_program(CC, baseA, baseB, NCHA, NCHB)

    xs_pad = np.zeros((NC, RTP, F), np.float32)
    xs_pad[:, :NPC] = X.reshape(NC, NPC, F)

    in_maps = [
        {
            "xs": xs_pad[i],
            "w1": W1f,
            "w2": W2f,
            "idxa": idxa[i],
            "idxb": idxb[i],
            "sa": sa[i],
            "sb": sb[i],
        }
        for i in range(NC)
    ]
    res = run_bass_kernel_spmd(
        nc, in_maps, core_ids=list(range(NC)), **(_run_kwargs or {})
    )
    out = np.concatenate([res.results[i]["out"][:NPC] for i in range(NC)], axis=0)
    if _run_kwargs:
        kernel.last_results = res
    return out



# revision 11
# speedup vs baseline: 1.2675x; 1.2675x over previous
"""EvolveGCN (2-layer) Trainium2 Bass kernel, 8-way sharded.

Algebraic reduction: the mat-GRU evolving the GCN weights is data-independent
and only h2[T-1] is returned, so the whole model collapses to

    W1* = matGRU^4(W1);  W2* = matGRU^4(W2)      (tiny host math)
    h1  = rrelu(A3 @ (X3 @ W1*));  out = rrelu(A3 @ (h1 @ W2*))

Device schedule (per core, nodes range-partitioned by original id):
  - X arrives transposed bf16 [128F, RTP]; table build is a plain matmul
    lhsT=xsT slice (even/odd row split so the fp16 DRAM shard writes are
    512B-contiguous), PSUM->fp16 via Activation copy.
  - AllGather replicates the fp16 table [50176, 128] to every core.
  - SWDGE dma_gather pulls per-edge messages (one 256B descriptor per edge)
    group A (table rows < 5*RTP) / group B split so indices fit int16.
  - Segment-sum runs on the tensor engine: per 64-row window, PSUM
    accumulates accT[128F, 64rows] += msg_chunk.T @ S_chunk, where S
    [128 edge-slots, 64 rows] carries val at (slot, row).  S is built
    on-device by the vector engine from packed val/rr arrays
    (S = (iota == rr) * val with 0-stride broadcast APs), not DMAed.
  - rrelu + down-cast is a single Prelu activation; layer-1 windows land in
    a transposed bf16 h1T tile that directly feeds the layer-2 table build
    (interleaved with layer-1's spmm); layer-2 windows land in a transposed
    fp16 out tile, written back per segment.
  - Host packs rows into windows (LPT on per-row A/B in-degree) so nearly
    every (window, group) hits its chunk budget exactly; the shared SPMD
    schedule is the per-window max over cores.
"""

import sys
import numpy as np

for _p in ("/opt/trn_rl_repo",):
    if _p not in sys.path:
        sys.path.insert(0, _p)

from ml_dtypes import bfloat16 as np_bf16

T, N, E, F = 4, 50000, 800000, 128
NC = 8
NPC = N // NC            # 6250 nodes per core
RTP = 6272               # padded rows per core (49 tiles of 128)
NT = RTP // 128          # 49 row tiles per core
WROWS = 64               # scatter window rows
NW = RTP // WROWS        # 98 windows per core
ACORES = 5               # table rows of cores [0,5) are group A
SPLIT = ACORES * RTP     # 31360 < 32768: both groups' indices fit int16
SLOPE = 11.0 / 48.0      # torch RReLU eval negative slope
SEGP = 2                 # row tiles per gather segment
TGT_A = 640              # per-window group-A edge target (5 chunks)
TGT_B = 384              # per-window group-B edge target (3 chunks)

SIM1 = False  # single-core, no-collective variant for TimelineSim
REPS = 1


def _evolve(W0, gW, gU, gb, steps=T):
    def sig(x):
        return 1.0 / (1.0 + np.exp(-x))

    Q = W0.astype(np.float64)
    gW = gW.astype(np.float64)
    gU = gU.astype(np.float64)
    gb = gb.astype(np.float64)
    for _ in range(steps):
        z = sig(gW[0] @ Q + gU[0] @ Q + gb[0])
        r = sig(gW[1] @ Q + gU[1] @ Q + gb[1])
        h = np.tanh(gW[2] @ Q + gU[2] @ (r * Q) + gb[2])
        Q = (1.0 - z) * Q + z * h
    return Q.astype(np.float32)


def _pack_windows(a, b, capA, capB):
    """Assign rows (with group in-degrees a, b) of one shard to NW windows of
    64 slots, keeping window sums <= (capA[w], capB[w]).  Snake-deal by
    degree, then swap-repair violations.  Returns positions in [0, RTP)."""
    n = len(a)
    order = np.argsort(-(a * 3 + b * 5), kind="stable")
    wins = np.empty(n, np.int64)
    # snake deal: balanced row counts and balanced degree sums
    for k, r in enumerate(order):
        rnd = k // NW
        j = k % NW
        wins[r] = j if rnd % 2 == 0 else NW - 1 - j
    A = np.bincount(wins, weights=a, minlength=NW).astype(np.int64)
    B = np.bincount(wins, weights=b, minlength=NW).astype(np.int64)
    members = [list(np.nonzero(wins == w)[0]) for w in range(NW)]

    stuck = np.zeros(NW, bool)
    for _it in range(8000):
        v = np.maximum(A - capA, 0) + np.maximum(B - capB, 0)
        v[stuck] = 0
        if v.max() == 0:
            if not stuck.any():
                break
            stuck[:] = False  # retry stuck ones once more
            v = np.maximum(A - capA, 0) + np.maximum(B - capB, 0)
            if v.max() == 0:
                break
        w = int(np.argmax(v))
        overA = A[w] > capA[w]
        overB = B[w] > capB[w]
        rows_w = members[w]
        sc = sorted(
            rows_w, key=lambda r: -(a[r] * overA + b[r] * overB)
        )[:10]
        roomA = capA - A
        roomB = capB - B
        done = False
        for r in sc:
            ar, br = a[r], b[r]
            cand = np.argsort(-(roomA + roomB))[:10]
            for w2 in cand:
                if w2 == w:
                    continue
                v_w = max(A[w] - capA[w], 0) + max(B[w] - capB[w], 0)
                v_2 = max(A[w2] - capA[w2], 0) + max(B[w2] - capB[w2], 0)
                for r2 in sorted(members[w2], key=lambda x: a[x] + b[x])[:10]:
                    a2, b2 = a[r2], b[r2]
                    nA_w, nB_w = A[w] - ar + a2, B[w] - br + b2
                    nA_2, nB_2 = A[w2] + ar - a2, B[w2] + br - b2
                    new = (max(nA_w - capA[w], 0) + max(nB_w - capB[w], 0)
                           + max(nA_2 - capA[w2], 0) + max(nB_2 - capB[w2], 0))
                    if new < v_w + v_2:
                        members[w].remove(r)
                        members[w2].remove(r2)
                        members[w].append(r2)
                        members[w2].append(r)
                        A[w], B[w] = nA_w, nB_w
                        A[w2], B[w2] = nA_2, nB_2
                        done = True
                        break
                if done:
                    break
            if done:
                break
        if not done:
            stuck[w] = True
            if stuck.all():
                break
    pos = np.empty(n, np.int64)
    for w in range(NW):
        pos[members[w]] = w * WROWS + np.arange(len(members[w]))
    return pos


def _prep_edges(row, col, val):
    """Host-side schedule. Returns (sched, per-core input arrays)."""
    # ---- window packing -> within-shard positions
    gcol = (col // NPC) >= ACORES
    a_deg = np.bincount(row[~gcol], minlength=N)
    b_deg = np.bincount(row[gcol], minlength=N)
    # shared overflow-window profile: last KA/KB windows get one extra chunk
    a_tot = a_deg.reshape(NC, NPC).sum(axis=1)
    b_tot = b_deg.reshape(NC, NPC).sum(axis=1)
    KA = max(0, -(-(int(a_tot.max()) + 256 - NW * TGT_A) // 128))
    KB = max(0, -(-(int(b_tot.max()) + 256 - NW * TGT_B) // 128))
    capA = np.full(NW, TGT_A, np.int64)
    capA[NW - KA :] = TGT_A + 128
    capB = np.full(NW, TGT_B, np.int64)
    capB[NW - KB :] = TGT_B + 128
    pos = np.empty(N, np.int64)
    for i in range(NC):
        lo, hi = i * NPC, (i + 1) * NPC
        pos[lo:hi] = _pack_windows(a_deg[lo:hi], b_deg[lo:hi], capA, capB)

    corei = row // NPC
    rl = pos[row]                       # scatter position within shard
    win = rl // WROWS
    rr = rl % WROWS
    tcol = (col // NPC) * RTP + pos[col]  # table row
    grp = (tcol >= SPLIT).astype(np.int64)

    # ---- merge exact duplicate (row, col) edges (S can only route a slot
    # to one destination row, so merging is valid only for identical rows)
    key = row * np.int64(N) + col
    order = np.argsort(key, kind="stable")
    key_s = key[order]
    uniq = np.empty(len(key_s), bool)
    uniq[0] = True
    uniq[1:] = key_s[1:] != key_s[:-1]
    seg_id = np.cumsum(uniq) - 1
    val_m = np.bincount(seg_id, weights=val[order].astype(np.float64))
    first = order[uniq]
    corei, win, rr, tcol, grp = (
        corei[first], win[first], rr[first], tcol[first], grp[first])
    val_m = val_m.astype(np.float32)

    # ---- shared chunk schedule: per (grp, win) max over cores
    counts = np.zeros((NC, 2, NW), np.int64)
    np.add.at(counts, (corei, grp, win), 1)
    CC = -(-counts // 128)
    CC = CC.max(axis=0)                 # [2, NW]
    CC[0] = np.maximum(CC[0], 1)        # every window needs >= 1 chunk
    baseA = np.zeros(NW + 1, np.int64)
    baseA[1:] = np.cumsum(CC[0])
    baseB = np.zeros(NW + 1, np.int64)
    baseB[1:] = np.cumsum(CC[1])
    NCHA, NCHB = int(baseA[-1]), int(baseB[-1])
    NCH = NCHA + NCHB
    NA, NB = NCHA * 128, NCHB * 128

    idxa = np.zeros((NC, 128, NA // 16), np.int16)
    idxb = np.zeros((NC, 128, NB // 16), np.int16)
    valp = np.zeros((NC, 128, NCH), np.float16)
    rrp = np.full((NC, 128, NCH), 127.0, np.float16)

    for i in range(NC):
        for g, (base, idxg, idxoff, choff) in enumerate(
            ((baseA, idxa, 0, 0), (baseB, idxb, SPLIT, NCHA))
        ):
            m = (corei == i) & (grp == g)
            ew, err = win[m], rr[m]
            etc = (tcol[m] - idxoff).astype(np.int16)
            ev = val_m[m]
            o = np.argsort(ew, kind="stable")
            ew, err, etc, ev = ew[o], err[o], etc[o], ev[o]
            winstart = np.searchsorted(ew, np.arange(NW))
            slot = base[ew] * 128 + (np.arange(ew.size) - winstart[ew])
            assert (slot < base[ew + 1] * 128).all()
            flat = np.zeros(base[-1] * 128, np.int16)
            flat[slot] = etc
            idxg[i][:16] = flat.reshape(-1, 16).T
            idxg[i] = np.tile(idxg[i][:16], (8, 1))
            p, ch = slot % 128, slot // 128 + choff
            valp[i, p, ch] = ev.astype(np.float16)
            rrp[i, p, ch] = err.astype(np.float16)

    sched = dict(CC=CC, baseA=baseA, baseB=baseB, NCHA=NCHA, NCHB=NCHB)
    return sched, pos, idxa, idxb, valp, rrp


def _build_program(sched):
    import concourse.bass as bass
    import concourse.tile as tile
    from concourse import bacc, mybir
    from contextlib import ExitStack

    F32, F16, BF16, I16 = (
        mybir.dt.float32, mybir.dt.float16, mybir.dt.bfloat16, mybir.dt.int16)
    baseA, baseB = sched["baseA"], sched["baseB"]
    NCHA, NCHB = sched["NCHA"], sched["NCHB"]
    NCH = NCHA + NCHB
    NA, NB = NCHA * 128, NCHB * 128

    nc = bacc.Bacc(
        "TRN2", target_bir_lowering=False, debug=False,
        num_devices=(1 if SIM1 else NC),
    )
    xst_d = nc.dram_tensor("xst", [F, RTP], BF16, kind="ExternalInput")
    w1_d = nc.dram_tensor("w1", [F, F], BF16, kind="ExternalInput")
    w2_d = nc.dram_tensor("w2", [F, F], BF16, kind="ExternalInput")
    iota_d = nc.dram_tensor("iota", [128, WROWS], F16, kind="ExternalInput")
    idxa_d = nc.dram_tensor("idxa", [128, NA // 16], I16, kind="ExternalInput")
    idxb_d = nc.dram_tensor("idxb", [128, NB // 16], I16, kind="ExternalInput")
    valp_d = nc.dram_tensor("valp", [128, NCH], F16, kind="ExternalInput")
    rrp_d = nc.dram_tensor("rrp", [128, NCH], F16, kind="ExternalInput")
    out_d = nc.dram_tensor("out", [F, RTP], F16, kind="ExternalOutput")

    # gather segments: SEGP row tiles each
    WQ = 128 // WROWS
    segs = []
    for p0 in range(0, NT, SEGP):
        p1 = min(p0 + SEGP, NT)
        segs.append((p0, p1, p0 * WQ, p1 * WQ))
    max_cha = max(int(baseA[w1] - baseA[w0]) for _, _, w0, w1 in segs)
    max_chb = max(int(baseB[w1] - baseB[w0]) for _, _, w0, w1 in segs)

    with tile.TileContext(nc) as tc, ExitStack() as ctx:
        const = ctx.enter_context(tc.tile_pool(name="const", bufs=1))
        big = ctx.enter_context(tc.tile_pool(name="big", bufs=1))
        tps = ctx.enter_context(tc.tile_pool(name="tps", bufs=2, space="PSUM"))
        tsh = ctx.enter_context(tc.tile_pool(name="tsh", bufs=4))
        accp = ctx.enter_context(tc.tile_pool(name="accp", bufs=6, space="PSUM"))
        msgp = ctx.enter_context(tc.tile_pool(name="msgp", bufs=3))
        h1p = ctx.enter_context(tc.tile_pool(name="h1p", bufs=3))
        dram = ctx.enter_context(tc.tile_pool(name="dram", bufs=1, space="DRAM"))

        # --- persistent SBUF tensors
        xst_sb = big.tile([F, RTP], BF16)
        nc.sync.dma_start(xst_sb[:], xst_d[:, :])
        w1_sb = const.tile([F, F], BF16)
        nc.sync.dma_start(w1_sb[:], w1_d[:, :])
        w2_sb = const.tile([F, F], BF16)
        nc.sync.dma_start(w2_sb[:], w2_d[:, :])

        def build_tiles(src_sb, src_t0, w_sb, shard, t0, t1):
            """table rows [t0*128, t1*128) = (src^T)[rows] @ w, written as
            [64, 256] fp16 tiles (rows 2p, 2p+1 on partition p).  src_sb
            holds tiles starting at global tile src_t0."""
            sh2 = shard.rearrange("(a b) -> a b", b=256)
            for t in range(t0, t1):
                s0 = (t - src_t0) * 128
                ps = tps.tile([64, 256], F32, tag="tp")
                for par in range(2):
                    nc.tensor.matmul(
                        out=ps[:, par * 128 : (par + 1) * 128],
                        lhsT=src_sb[:, s0 + par : s0 + 128 : 2],
                        rhs=w_sb[:],
                        start=True, stop=True,
                    )
                sh = tsh.tile([64, 256], F16, tag="sh")
                nc.scalar.activation(
                    sh[:], ps[:], mybir.ActivationFunctionType.Copy
                )
                nc.sync.dma_start(sh2[t * 64 : (t + 1) * 64, :], sh[:])

        def all_gather(shard, table):
            if SIM1:
                for r in range(NC):
                    nc.sync.dma_start(
                        table[r * RTP * F : (r + 1) * RTP * F], shard[:]
                    )
            else:
                nc.gpsimd.collective_compute(
                    "AllGather",
                    mybir.AluOpType.bypass,
                    replica_groups=[list(range(NC))],
                    ins=[shard.opt()],
                    outs=[table.opt()],
                )

        # --- layer-1 table build + allgather
        _aspace = "Local" if SIM1 else "Shared"
        shard1 = dram.tile([RTP * F], F16, name="shard1")
        shard2 = dram.tile([RTP * F], F16, name="shard2")
        table1 = dram.tile([NC * RTP * F], F16, addr_space=_aspace, name="table1")
        table2 = dram.tile([NC * RTP * F], F16, addr_space=_aspace, name="table2")
        build_tiles(xst_sb, 0, w1_sb, shard1, 0, NT)
        all_gather(shard1, table1)

        # --- S built on DVE: S[p, cid*64 + j] = (iota[j] == rr[p,cid]) * val
        iota_sb = const.tile([128, WROWS], F16)
        nc.sync.dma_start(iota_sb[:], iota_d[:, :])
        idxa_sb = big.tile([128, NA // 16], I16)
        nc.sync.dma_start(idxa_sb[:], idxa_d[:, :])
        idxb_sb = big.tile([128, NB // 16], I16)
        nc.sync.dma_start(idxb_sb[:], idxb_d[:, :])
        valp_sb = big.tile([128, NCH], F16)
        nc.sync.dma_start(valp_sb[:], valp_d[:, :])
        rrp_sb = big.tile([128, NCH], F16)
        nc.sync.dma_start(rrp_sb[:], rrp_d[:, :])
        s_sb = big.tile([128, NCH * WROWS], F16)
        SLAB = 128
        for c0 in range(0, NCH, SLAB):
            c1 = min(c0 + SLAB, NCH)
            nch = c1 - c0
            s_slab = s_sb[:, c0 * WROWS : c1 * WROWS]
            s3 = s_slab.rearrange("p (c j) -> p c j", j=WROWS)
            iota_b = iota_sb[:, :].unsqueeze(1).broadcast_to([128, nch, WROWS])
            rr_b = rrp_sb[:, c0:c1].unsqueeze(2).broadcast_to([128, nch, WROWS])
            val_b = valp_sb[:, c0:c1].unsqueeze(2).broadcast_to([128, nch, WROWS])
            nc.vector.tensor_tensor(
                out=s3, in0=iota_b, in1=rr_b, op=mybir.AluOpType.is_equal
            )
            nc.vector.tensor_tensor(
                out=s3, in0=s3, in1=val_b, op=mybir.AluOpType.mult
            )

        def spmm(table, emit, interleave=None):
            tbl = table.rearrange("(r f) -> r f", f=F)
            for si, (p0, p1, w0, w1) in enumerate(segs):
                ca0, ca1 = int(baseA[w0]), int(baseA[w1])
                cb0, cb1 = int(baseB[w0]), int(baseB[w1])
                na, nb = (ca1 - ca0) * 128, (cb1 - cb0) * 128
                msga = msgp.tile([128, max_cha, 128], F16, tag="msga")
                msgb = msgp.tile([128, max_chb, 128], F16, tag="msgb")
                if na:
                    nc.gpsimd.dma_gather(
                        out_ap=msga[:, : ca1 - ca0, :],
                        in_ap=tbl[:SPLIT, :],
                        idxs_ap=idxa_sb[:, ca0 * 8 : ca1 * 8],
                        num_idxs=na,
                        num_idxs_reg=na,
                        elem_size=F,
                        single_packet=False,
                    )
                if nb:
                    nc.gpsimd.dma_gather(
                        out_ap=msgb[:, : cb1 - cb0, :],
                        in_ap=tbl[SPLIT:, :],
                        idxs_ap=idxb_sb[:, cb0 * 8 : cb1 * 8],
                        num_idxs=nb,
                        num_idxs_reg=nb,
                        elem_size=F,
                        single_packet=False,
                    )
                emt = emit(si)
                for w in range(w0, w1):
                    acc = accp.tile([128, WROWS], F32, tag="acc")
                    nw_ch = int(
                        baseA[w + 1] - baseA[w] + baseB[w + 1] - baseB[w]
                    )
                    k = 0
                    for gc in range(int(baseA[w]), int(baseA[w + 1])):
                        nc.tensor.matmul(
                            out=acc[:],
                            lhsT=msga[:, gc - ca0, :],
                            rhs=s_sb[:, gc * WROWS : (gc + 1) * WROWS],
                            start=(k == 0),
                            stop=(k == nw_ch - 1),
                        )
                        k += 1
                    for gc in range(int(baseB[w]), int(baseB[w + 1])):
                        cid = NCHA + gc
                        nc.tensor.matmul(
                            out=acc[:],
                            lhsT=msgb[:, gc - cb0, :],
                            rhs=s_sb[:, cid * WROWS : (cid + 1) * WROWS],
                            start=(k == 0),
                            stop=(k == nw_ch - 1),
                        )
                        k += 1
                    emt(w - w0, acc)
                if interleave is not None:
                    interleave(si, p0, p1)

        # --- layer 1: spmm -> h1T (bf16, per-segment tiles) -> table2 build
        h1tiles = {}

        def emit1(si):
            h1t = h1p.tile([F, SEGP * 128], BF16, tag="h1t")
            h1tiles[si] = h1t

            def e(wloc, acc):
                nc.scalar.activation(
                    h1t[:, wloc * WROWS : (wloc + 1) * WROWS],
                    acc[:],
                    mybir.ActivationFunctionType.Prelu,
                    alpha=SLOPE,
                )
            return e

        def interleave1(si, p0, p1):
            build_tiles(h1tiles[si], p0, w2_sb, shard2, p0, p1)

        spmm(table1, emit1, interleave1)
        all_gather(shard2, table2)

        # --- layer 2: spmm -> outT fp16 -> DRAM per segment
        out_sb = big.tile([F, RTP], F16)

        def emit2(si):
            p0, p1, w0, w1 = segs[si]

            def e(wloc, acc):
                w = w0 + wloc
                nc.scalar.activation(
                    out_sb[:, w * WROWS : (w + 1) * WROWS],
                    acc[:],
                    mybir.ActivationFunctionType.Prelu,
                    alpha=SLOPE,
                )
            return e

        def interleave2(si, p0, p1):
            nc.sync.dma_start(
                out_d[:, p0 * 128 : p1 * 128],
                out_sb[:, p0 * 128 : p1 * 128],
            )

        spmm(table2, emit2, interleave2)

    nc.compile()
    return nc


def kernel(
    features,
    adj_row,
    adj_col,
    adj_val,
    W1,
    g1_W,
    g1_U,
    g1_b,
    W2,
    g2_W,
    g2_U,
    g2_b,
    _run_kwargs=None,
):
    from concourse.bass_utils import run_bass_kernel_spmd

    X = np.asarray(features[T - 1], dtype=np.float32)
    row = np.asarray(adj_row[T - 1], dtype=np.int64)
    col = np.asarray(adj_col[T - 1], dtype=np.int64)
    val = np.asarray(adj_val[T - 1], dtype=np.float32)

    W1f = _evolve(np.asarray(W1), np.asarray(g1_W), np.asarray(g1_U), np.asarray(g1_b))
    W2f = _evolve(np.asarray(W2), np.asarray(g2_W), np.asarray(g2_U), np.asarray(g2_b))

    sched, pos, idxa, idxb, valp, rrp = _prep_edges(row, col, val)
    nc = _build_program(sched)

    # xsT per core: [128, RTP] bf16, column pos[v] = X[v]
    xst = np.zeros((NC, F, RTP), np_bf16)
    for i in range(NC):
        lo, hi = i * NPC, (i + 1) * NPC
        xst[i][:, pos[lo:hi]] = X[lo:hi].T.astype(np_bf16)

    iota = np.broadcast_to(
        np.arange(WROWS, dtype=np.float16), (128, WROWS)
    ).copy()

    in_maps = [
        {
            "xst": xst[i],
            "w1": W1f.astype(np_bf16),
            "w2": W2f.astype(np_bf16),
            "iota": iota,
            "idxa": idxa[i],
            "idxb": idxb[i],
            "valp": valp[i],
            "rrp": rrp[i],
        }
        for i in range(NC)
    ]
    res = run_bass_kernel_spmd(
        nc, in_maps, core_ids=list(range(NC)), **(_run_kwargs or {})
    )
    out = np.empty((N, F), np.float32)
    for i in range(NC):
        lo, hi = i * NPC, (i + 1) * NPC
        arr = res.results[i]["out"].astype(np.float32)  # [F, RTP]
        out[lo:hi] = arr[:, pos[lo:hi]].T
    if _run_kwargs:
        kernel.last_results = res
    return out


# revision 26
# speedup vs baseline: 1.3435x; 1.0599x over previous
"""EvolveGCN (2-layer) Trainium2 Bass kernel, 8-way sharded.

Algebraic reduction: the mat-GRU evolving the GCN weights is data-independent
and only h2[T-1] is returned, so the whole model collapses to

    W1* = matGRU^4(W1);  W2* = matGRU^4(W2)      (tiny host math)
    h1  = rrelu(A3 @ (X3 @ W1*));  out = rrelu(A3 @ (h1 @ W2*))

Device schedule (per core, nodes range-partitioned by original id):
  - X arrives transposed bf16 [128F, RTP]; table build is a plain matmul
    lhsT=xsT slice (even/odd row split so the fp16 DRAM shard writes are
    512B-contiguous), PSUM->fp16 via Activation copy.
  - AllGather replicates the fp16 table [50176, 128] to every core.
  - SWDGE dma_gather pulls per-edge messages (one 256B descriptor per edge)
    group A (table rows < 5*RTP) / group B split so indices fit int16.
  - Segment-sum runs on the tensor engine: per 64-row window, PSUM
    accumulates accT[128F, 64rows] += msg_chunk.T @ S_chunk, where S
    [128 edge-slots, 64 rows] carries val at (slot, row).  S is built
    on-device by the vector engine from packed val/rr arrays
    (S = (iota == rr) * val with 0-stride broadcast APs), not DMAed.
  - rrelu + down-cast is a single Prelu activation; layer-1 windows land in
    a transposed bf16 h1T tile that directly feeds the layer-2 table build
    (interleaved with layer-1's spmm); layer-2 windows land in a transposed
    fp16 out tile, written back per segment.
  - Host packs rows into windows (LPT on per-row A/B in-degree) so nearly
    every (window, group) hits its chunk budget exactly; the shared SPMD
    schedule is the per-window max over cores.
"""

import sys
import numpy as np

for _p in ("/opt/trn_rl_repo",):
    if _p not in sys.path:
        sys.path.insert(0, _p)

from ml_dtypes import bfloat16 as np_bf16

T, N, E, F = 4, 50000, 800000, 128
NC = 8
NPC = N // NC            # 6250 nodes per core
RTP = 6272               # padded rows per core (49 tiles of 128)
NT = RTP // 128          # 49 row tiles per core
WROWS = 64               # scatter window rows
NW = RTP // WROWS        # 98 windows per core
ACORES = 5               # table rows of cores [0,5) are group A
SPLIT = ACORES * RTP     # 31360 < 32768: both groups' indices fit int16
SLOPE = 11.0 / 48.0      # torch RReLU eval negative slope
SEGP = 2                 # row tiles per gather segment
TGT_A = 640              # per-window group-A edge target (5 chunks)
TGT_B = 384              # per-window group-B edge target (3 chunks)

SIM1 = False  # single-core, no-collective variant for TimelineSim
REPS = 1


def _evolve(W0, gW, gU, gb, steps=T):
    def sig(x):
        return 1.0 / (1.0 + np.exp(-x))

    Q = W0.astype(np.float64)
    gW = gW.astype(np.float64)
    gU = gU.astype(np.float64)
    gb = gb.astype(np.float64)
    for _ in range(steps):
        z = sig(gW[0] @ Q + gU[0] @ Q + gb[0])
        r = sig(gW[1] @ Q + gU[1] @ Q + gb[1])
        h = np.tanh(gW[2] @ Q + gU[2] @ (r * Q) + gb[2])
        Q = (1.0 - z) * Q + z * h
    return Q.astype(np.float32)


def _pack_windows(a, b, capA, capB, rng):
    """Assign rows (with group in-degrees a, b) of one shard to NW windows of
    64 slots, keeping window sums <= (capA[w], capB[w]).  Snake-deal by
    degree, then pairwise swap-repair of violations.  Returns positions."""
    n = len(a)
    order = np.argsort(-(a * 3 + b * 5), kind="stable")
    wins = np.empty(n, np.int64)
    rnds = np.arange(n) // NW
    js = np.arange(n) % NW
    wins[order] = np.where(rnds % 2 == 0, js, NW - 1 - js)

    def sums():
        A = np.bincount(wins, weights=a, minlength=NW).astype(np.int64)
        B = np.bincount(wins, weights=b, minlength=NW).astype(np.int64)
        return A, B

    A, B = sums()
    members = [list(np.nonzero(wins == w)[0]) for w in range(NW)]
    al = a.tolist()
    bl = b.tolist()
    capAl, capBl = capA.tolist(), capB.tolist()
    stuck = np.zeros(NW, bool)
    resets = 0
    for _it in range(6000):
        vA = np.maximum(A - capA, 0)
        vB = np.maximum(B - capB, 0)
        v = vA + vB
        va = v.copy()
        va[stuck] = 0
        if va.max() == 0:
            if v.max() == 0 or stuck.all() or resets >= 2:
                break
            stuck[:] = False
            resets += 1
            continue
        w = int(np.argmax(va))
        overA = bool(vA[w] > 0)
        overB = bool(vB[w] > 0)
        rw = members[w]
        sc_w = sorted(rw, key=lambda r: -(al[r] * overA + bl[r] * overB))[:10]
        roomA = capA - A
        roomB = capB - B
        cand_w2 = np.argpartition(-(roomA + roomB), 10)[:10]
        cand_w2 = cand_w2[np.argsort(-(roomA + roomB)[cand_w2])]
        done = False
        for r in sc_w:
            ar, br = al[r], bl[r]
            for w2 in cand_w2:
                if w2 == w:
                    continue
                w2 = int(w2)
                r2i = sorted(
                    members[w2],
                    key=lambda x: al[x] * overA + bl[x] * overB,
                )[:10]
                vold = int(v[w] + v[w2])
                for r2 in r2i:
                    a2, b2 = al[r2], bl[r2]
                    nA_w, nB_w = A[w] - ar + a2, B[w] - br + b2
                    nA_2, nB_2 = A[w2] + ar - a2, B[w2] + br - b2
                    new = (max(nA_w - capAl[w], 0) + max(nB_w - capBl[w], 0)
                           + max(nA_2 - capAl[w2], 0) + max(nB_2 - capBl[w2], 0))
                    if new < vold:
                        wins[r], wins[r2] = w2, w
                        members[w].remove(r)
                        members[w2].remove(r2)
                        members[w].append(r2)
                        members[w2].append(r)
                        A[w], B[w] = nA_w, nB_w
                        A[w2], B[w2] = nA_2, nB_2
                        done = True
                        break
                if done:
                    break
            if done:
                break
        if not done:
            stuck[w] = True
    pos = np.empty(n, np.int64)
    for w in range(NW):
        rows = np.nonzero(wins == w)[0]
        pos[rows] = w * WROWS + np.arange(len(rows))
    return pos


def _prep_edges(row, col, val):
    """Host-side schedule. Returns (sched, per-core input arrays)."""
    # ---- window packing -> within-shard positions
    gcol = (col // NPC) >= ACORES
    a_deg = np.bincount(row[~gcol], minlength=N)
    b_deg = np.bincount(row[gcol], minlength=N)
    # shared overflow-window profile: last KA/KB windows get one extra chunk
    a_tot = a_deg.reshape(NC, NPC).sum(axis=1)
    b_tot = b_deg.reshape(NC, NPC).sum(axis=1)
    KA = max(0, -(-(int(a_tot.max()) + 256 - NW * TGT_A) // 128))
    KB = max(0, -(-(int(b_tot.max()) + 256 - NW * TGT_B) // 128))
    capA = np.full(NW, TGT_A, np.int64)
    capA[NW - KA :] = TGT_A + 128
    capB = np.full(NW, TGT_B, np.int64)
    capB[NW - KB :] = TGT_B + 128
    pos = np.empty(N, np.int64)
    rng = np.random.default_rng(0)
    for i in range(NC):
        lo, hi = i * NPC, (i + 1) * NPC
        pos[lo:hi] = _pack_windows(a_deg[lo:hi], b_deg[lo:hi], capA, capB, rng)

    corei = row // NPC
    rl = pos[row]                       # scatter position within shard
    win = rl // WROWS
    rr = rl % WROWS
    tcol = (col // NPC) * RTP + pos[col]  # table row
    grp = (tcol >= SPLIT).astype(np.int64)

    # ---- merge exact duplicate (row, col) edges (S can only route a slot
    # to one destination row, so merging is valid only for identical rows)
    key = row * np.int64(N) + col
    order = np.argsort(key, kind="stable")
    key_s = key[order]
    uniq = np.empty(len(key_s), bool)
    uniq[0] = True
    uniq[1:] = key_s[1:] != key_s[:-1]
    seg_id = np.cumsum(uniq) - 1
    val_m = np.bincount(seg_id, weights=val[order].astype(np.float64))
    first = order[uniq]
    corei, win, rr, tcol, grp = (
        corei[first], win[first], rr[first], tcol[first], grp[first])
    val_m = val_m.astype(np.float32)

    # ---- shared chunk schedule: per (grp, win) max over cores
    counts = np.zeros((NC, 2, NW), np.int64)
    np.add.at(counts, (corei, grp, win), 1)
    CC = -(-counts // 128)
    CC = CC.max(axis=0)                 # [2, NW]
    CC[0] = np.maximum(CC[0], 1)        # every window needs >= 1 chunk
    baseA = np.zeros(NW + 1, np.int64)
    baseA[1:] = np.cumsum(CC[0])
    baseB = np.zeros(NW + 1, np.int64)
    baseB[1:] = np.cumsum(CC[1])
    NCHA, NCHB = int(baseA[-1]), int(baseB[-1])
    NCH = NCHA + NCHB
    NA, NB = NCHA * 128, NCHB * 128
    # unified S chunk ids, window-major (A then B within each window) so the
    # DVE S-build completes chunks in the order the spmm consumes them
    offW = np.zeros(NW + 1, np.int64)
    offW[1:] = np.cumsum(CC[0] + CC[1])

    idxa = np.zeros((NC, 128, NA // 16), np.int16)
    idxb = np.zeros((NC, 128, NB // 16), np.int16)
    valp = np.zeros((NC, 128, NCH), np.float16)
    rrp = np.full((NC, 128, NCH), 127.0, np.float16)

    for i in range(NC):
        for g, (base, idxg, idxoff) in enumerate(
            ((baseA, idxa, 0), (baseB, idxb, SPLIT))
        ):
            m = (corei == i) & (grp == g)
            ew, err = win[m], rr[m]
            etc = (tcol[m] - idxoff).astype(np.int16)
            ev = val_m[m]
            o = np.argsort(ew, kind="stable")
            ew, err, etc, ev = ew[o], err[o], etc[o], ev[o]
            winstart = np.searchsorted(ew, np.arange(NW))
            slot = base[ew] * 128 + (np.arange(ew.size) - winstart[ew])
            assert (slot < base[ew + 1] * 128).all()
            flat = np.zeros(base[-1] * 128, np.int16)
            flat[slot] = etc
            idxg[i][:16] = flat.reshape(-1, 16).T
            idxg[i] = np.tile(idxg[i][:16], (8, 1))
            p = slot % 128
            # unified chunk id: window-major
            gch = slot // 128                    # group-major chunk id
            loc = gch - base[ew]                 # chunk within window
            ch = offW[ew] + g * CC[0][ew] + loc
            valp[i, p, ch] = ev.astype(np.float16)
            rrp[i, p, ch] = err.astype(np.float16)

    sched = dict(
        CC=CC, baseA=baseA, baseB=baseB, NCHA=NCHA, NCHB=NCHB, offW=offW
    )
    return sched, pos, idxa, idxb, valp, rrp


def _build_program(sched):
    import concourse.bass as bass
    import concourse.tile as tile
    from concourse import bacc, mybir
    from contextlib import ExitStack

    F32, F16, BF16, I16 = (
        mybir.dt.float32, mybir.dt.float16, mybir.dt.bfloat16, mybir.dt.int16)
    baseA, baseB = sched["baseA"], sched["baseB"]
    NCHA, NCHB = sched["NCHA"], sched["NCHB"]
    offW = sched["offW"]
    CCA = sched["CC"][0]
    NCH = NCHA + NCHB
    NA, NB = NCHA * 128, NCHB * 128

    nc = bacc.Bacc(
        "TRN2", target_bir_lowering=False, debug=False,
        num_devices=(1 if SIM1 else NC),
    )
    xst_d = nc.dram_tensor("xst", [F, RTP], BF16, kind="ExternalInput")
    w1_d = nc.dram_tensor("w1", [F, F], BF16, kind="ExternalInput")
    w2_d = nc.dram_tensor("w2", [F, F], BF16, kind="ExternalInput")
    iota_d = nc.dram_tensor("iota", [128, WROWS], F16, kind="ExternalInput")
    idxa_d = nc.dram_tensor("idxa", [128, NA // 16], I16, kind="ExternalInput")
    idxb_d = nc.dram_tensor("idxb", [128, NB // 16], I16, kind="ExternalInput")
    valp_d = nc.dram_tensor("valp", [128, NCH], F16, kind="ExternalInput")
    rrp_d = nc.dram_tensor("rrp", [128, NCH], F16, kind="ExternalInput")
    out_d = nc.dram_tensor("out", [F, RTP], F16, kind="ExternalOutput")

    # gather segments: SEGP row tiles each
    WQ = 128 // WROWS
    segs = []
    for p0 in range(0, NT, SEGP):
        p1 = min(p0 + SEGP, NT)
        segs.append((p0, p1, p0 * WQ, p1 * WQ))
    max_cha = max(int(baseA[w1] - baseA[w0]) for _, _, w0, w1 in segs)
    max_chb = max(int(baseB[w1] - baseB[w0]) for _, _, w0, w1 in segs)

    with tile.TileContext(nc) as tc, ExitStack() as ctx:
        const = ctx.enter_context(tc.tile_pool(name="const", bufs=1))
        big = ctx.enter_context(tc.tile_pool(name="big", bufs=1))
        tps = ctx.enter_context(tc.tile_pool(name="tps", bufs=2, space="PSUM"))
        tsh = ctx.enter_context(tc.tile_pool(name="tsh", bufs=4))
        accp = ctx.enter_context(tc.tile_pool(name="accp", bufs=4, space="PSUM"))
        msgp = ctx.enter_context(tc.tile_pool(name="msgp", bufs=3))
        h1p = ctx.enter_context(tc.tile_pool(name="h1p", bufs=3))
        dram = ctx.enter_context(tc.tile_pool(name="dram", bufs=1, space="DRAM"))

        # table-build / allgather chunks (tile ranges), segment-aligned,
        # small tail chunk so the layer transition drains fast
        CHB = [0, 10, 20, 28, 36, 44, NT]
        NCHK = len(CHB) - 1

        # --- inputs with no deps first: fill DMA idle during table build
        w1_sb = const.tile([F, F], BF16)
        nc.sync.dma_start(w1_sb[:], w1_d[:, :])
        w2_sb = const.tile([F, F], BF16)
        nc.sync.dma_start(w2_sb[:], w2_d[:, :])
        xst_c = []
        for g in range(NCHK):
            t0, t1 = CHB[g], CHB[g + 1]
            xt = big.tile([F, (t1 - t0) * 128], BF16, name=f"xst{g}")
            nc.sync.dma_start(xt[:], xst_d[:, t0 * 128 : t1 * 128])
            xst_c.append(xt)
        iota_sb = const.tile([128, WROWS], F16)
        nc.sync.dma_start(iota_sb[:], iota_d[:, :])
        idxa_sb = big.tile([128, NA // 16], I16)
        nc.sync.dma_start(idxa_sb[:], idxa_d[:, :])
        idxb_sb = big.tile([128, NB // 16], I16)
        nc.sync.dma_start(idxb_sb[:], idxb_d[:, :])
        valp_sb = big.tile([128, NCH], F16)
        nc.sync.dma_start(valp_sb[:], valp_d[:, :])
        rrp_sb = big.tile([128, NCH], F16)
        nc.sync.dma_start(rrp_sb[:], rrp_d[:, :])

        # --- S built on DVE: S[p, cid*64 + j] = (iota[j] == rr[p,cid]) * val
        s_sb = big.tile([128, NCH * WROWS], F16)
        SLAB = 128
        for c0 in range(0, NCH, SLAB):
            c1 = min(c0 + SLAB, NCH)
            nch = c1 - c0
            s_slab = s_sb[:, c0 * WROWS : c1 * WROWS]
            s3 = s_slab.rearrange("p (c j) -> p c j", j=WROWS)
            iota_b = iota_sb[:, :].unsqueeze(1).broadcast_to([128, nch, WROWS])
            rr_b = rrp_sb[:, c0:c1].unsqueeze(2).broadcast_to([128, nch, WROWS])
            val_b = valp_sb[:, c0:c1].unsqueeze(2).broadcast_to([128, nch, WROWS])
            nc.vector.tensor_tensor(
                out=s3, in0=iota_b, in1=rr_b, op=mybir.AluOpType.is_equal
            )
            nc.vector.tensor_tensor(
                out=s3, in0=s3, in1=val_b, op=mybir.AluOpType.mult
            )

        def build_tiles(src_sb, src_t0, w_sb, shard, t0, t1):
            """table rows [t0*128, t1*128) = (src^T)[rows] @ w, written as
            [64, 256]-per-tile fp16 (rows 2p, 2p+1 on partition p), up to
            four tiles per activation/DMA."""
            sh3 = shard.rearrange("(t q b) -> q t b", q=64, b=256)
            t = t0
            while t < t1:
                grp = min(4, t1 - t)
                ps = tps.tile([64, 1024], F32, tag="tp")
                for k in range(grp):
                    s0 = (t + k - src_t0) * 128
                    for par in range(2):
                        nc.tensor.matmul(
                            out=ps[:, k * 256 + par * 128 : k * 256 + (par + 1) * 128],
                            lhsT=src_sb[:, s0 + par : s0 + 128 : 2],
                            rhs=w_sb[:],
                            start=True, stop=True,
                        )
                sh = tsh.tile([64, 1024], F16, tag="sh")
                nc.scalar.activation(
                    sh[:, : grp * 256], ps[:, : grp * 256],
                    mybir.ActivationFunctionType.Copy,
                )
                sh_t = sh.rearrange("p (t b) -> p t b", b=256)
                nc.sync.dma_start(
                    sh3[:, t : t + grp, :],
                    sh_t[:, :grp, :],
                )
                t += grp

        def all_gather(shard, table):
            if SIM1:
                for r in range(NC):
                    nc.sync.dma_start(
                        table[r * RTP * F : (r + 1) * RTP * F], shard[:]
                    )
            else:
                nc.gpsimd.collective_compute(
                    "AllGather",
                    mybir.AluOpType.bypass,
                    replica_groups=[list(range(NC))],
                    ins=[shard.opt()],
                    outs=[table.opt()],
                )

        # --- layer-1 table build (chunked for pipelining) + allgather
        _aspace = "Local" if SIM1 else "Shared"
        shard1 = dram.tile([RTP * F], F16, name="shard1")
        shard2 = dram.tile([RTP * F], F16, name="shard2")
        table1 = dram.tile([NC * RTP * F], F16, addr_space=_aspace, name="table1")
        table2 = dram.tile([NC * RTP * F], F16, addr_space=_aspace, name="table2")
        for g in range(NCHK):
            build_tiles(xst_c[g], CHB[g], w1_sb, shard1, CHB[g], CHB[g + 1])
        all_gather(shard1, table1)

        def spmm(table, emit, interleave=None):
            tbl = table.rearrange("(r f) -> r f", f=F)
            for si, (p0, p1, w0, w1) in enumerate(segs):
                ca0, ca1 = int(baseA[w0]), int(baseA[w1])
                cb0, cb1 = int(baseB[w0]), int(baseB[w1])
                na, nb = (ca1 - ca0) * 128, (cb1 - cb0) * 128
                msga = msgp.tile([128, max_cha, 128], F16, tag="msga")
                msgb = msgp.tile([128, max_chb, 128], F16, tag="msgb")
                if na:
                    nc.gpsimd.dma_gather(
                        out_ap=msga[:, : ca1 - ca0, :],
                        in_ap=tbl[:SPLIT, :],
                        idxs_ap=idxa_sb[:, ca0 * 8 : ca1 * 8],
                        num_idxs=na,
                        num_idxs_reg=na,
                        elem_size=F,
                        single_packet=False,
                    )
                if nb:
                    nc.gpsimd.dma_gather(
                        out_ap=msgb[:, : cb1 - cb0, :],
                        in_ap=tbl[SPLIT:, :],
                        idxs_ap=idxb_sb[:, cb0 * 8 : cb1 * 8],
                        num_idxs=nb,
                        num_idxs_reg=nb,
                        elem_size=F,
                        single_packet=False,
                    )
                emt = emit(si)
                for w in range(w0, w1):
                    acc = accp.tile([128, WROWS], F32, tag="acc")
                    nw_ch = int(
                        baseA[w + 1] - baseA[w] + baseB[w + 1] - baseB[w]
                    )
                    k = 0
                    for gc in range(int(baseA[w]), int(baseA[w + 1])):
                        cid = int(offW[w]) + (gc - int(baseA[w]))
                        nc.tensor.matmul(
                            out=acc[:],
                            lhsT=msga[:, gc - ca0, :],
                            rhs=s_sb[:, cid * WROWS : (cid + 1) * WROWS],
                            start=(k == 0),
                            stop=(k == nw_ch - 1),
                        )
                        k += 1
                    for gc in range(int(baseB[w]), int(baseB[w + 1])):
                        cid = int(offW[w]) + int(CCA[w]) + (gc - int(baseB[w]))
                        nc.tensor.matmul(
                            out=acc[:],
                            lhsT=msgb[:, gc - cb0, :],
                            rhs=s_sb[:, cid * WROWS : (cid + 1) * WROWS],
                            start=(k == 0),
                            stop=(k == nw_ch - 1),
                        )
                        k += 1
                    emt(w - w0, acc)
                if interleave is not None:
                    interleave(si, p0, p1)

        # --- layer 1: spmm -> h1T (bf16, per-segment tiles) -> table2 build
        h1tiles = {}

        def emit1(si):
            h1t = h1p.tile([F, SEGP * 128], BF16, tag="h1t")
            h1tiles[si] = h1t

            def e(wloc, acc):
                nc.scalar.activation(
                    h1t[:, wloc * WROWS : (wloc + 1) * WROWS],
                    acc[:],
                    mybir.ActivationFunctionType.Prelu,
                    alpha=SLOPE,
                )
            return e

        def interleave1(si, p0, p1):
            build_tiles(h1tiles[si], p0, w2_sb, shard2, p0, p1)

        spmm(table1, emit1, interleave1)
        all_gather(shard2, table2)

        # --- layer 2: spmm -> outT fp16 -> DRAM per segment
        out_sb = big.tile([F, RTP], F16)

        def emit2(si):
            p0, p1, w0, w1 = segs[si]

            def e(wloc, acc):
                w = w0 + wloc
                nc.scalar.activation(
                    out_sb[:, w * WROWS : (w + 1) * WROWS],
                    acc[:],
                    mybir.ActivationFunctionType.Prelu,
                    alpha=SLOPE,
                )
            return e

        def interleave2(si, p0, p1):
            nc.sync.dma_start(
                out_d[:, p0 * 128 : p1 * 128],
                out_sb[:, p0 * 128 : p1 * 128],
            )

        spmm(table2, emit2, interleave2)

    nc.compile()
    return nc


def kernel(
    features,
    adj_row,
    adj_col,
    adj_val,
    W1,
    g1_W,
    g1_U,
    g1_b,
    W2,
    g2_W,
    g2_U,
    g2_b,
    _run_kwargs=None,
):
    from concourse.bass_utils import run_bass_kernel_spmd

    X = np.asarray(features[T - 1], dtype=np.float32)
    row = np.asarray(adj_row[T - 1], dtype=np.int64)
    col = np.asarray(adj_col[T - 1], dtype=np.int64)
    val = np.asarray(adj_val[T - 1], dtype=np.float32)

    W1f = _evolve(np.asarray(W1), np.asarray(g1_W), np.asarray(g1_U), np.asarray(g1_b))
    W2f = _evolve(np.asarray(W2), np.asarray(g2_W), np.asarray(g2_U), np.asarray(g2_b))

    sched, pos, idxa, idxb, valp, rrp = _prep_edges(row, col, val)
    nc = _build_program(sched)

    # xsT per core: [128, RTP] bf16, column pos[v] = X[v]
    xst = np.zeros((NC, F, RTP), np_bf16)
    for i in range(NC):
        lo, hi = i * NPC, (i + 1) * NPC
        xst[i][:, pos[lo:hi]] = X[lo:hi].T.astype(np_bf16)

    iota = np.broadcast_to(
        np.arange(WROWS, dtype=np.float16), (128, WROWS)
    ).copy()

    in_maps = [
        {
            "xst": xst[i],
            "w1": W1f.astype(np_bf16),
            "w2": W2f.astype(np_bf16),
            "iota": iota,
            "idxa": idxa[i],
            "idxb": idxb[i],
            "valp": valp[i],
            "rrp": rrp[i],
        }
        for i in range(NC)
    ]
    res = run_bass_kernel_spmd(
        nc, in_maps, core_ids=list(range(NC)), **(_run_kwargs or {})
    )
    out = np.empty((N, F), np.float32)
    for i in range(NC):
        lo, hi = i * NPC, (i + 1) * NPC
        arr = res.results[i]["out"].astype(np.float32)  # [F, RTP]
        out[lo:hi] = arr[:, pos[lo:hi]].T
    if _run_kwargs:
        kernel.last_results = res
    return out


# revision 43
# speedup vs baseline: 1.3592x; 1.0117x over previous
"""EvolveGCN (2-layer) Trainium2 Bass kernel, 8-way sharded.

Algebraic reduction: the mat-GRU evolving the GCN weights is data-independent
and only h2[T-1] is returned, so the whole model collapses to

    W1* = matGRU^4(W1);  W2* = matGRU^4(W2)      (tiny host math)
    h1  = rrelu(A3 @ (X3 @ W1*));  out = rrelu(A3 @ (h1 @ W2*))

Device schedule (per core, nodes range-partitioned by original id):
  - X arrives transposed bf16 [128F, RTP]; table build is a plain matmul
    lhsT=xsT slice (even/odd row split so the fp16 DRAM shard writes are
    512B-contiguous), PSUM->fp16 via Activation copy.
  - AllGather replicates the fp16 table [50176, 128] to every core.
  - SWDGE dma_gather pulls per-edge messages (one 256B descriptor per edge)
    group A (table rows < 5*RTP) / group B split so indices fit int16.
  - Segment-sum runs on the tensor engine: per 64-row window, PSUM
    accumulates accT[128F, 64rows] += msg_chunk.T @ S_chunk, where S
    [128 edge-slots, 64 rows] carries val at (slot, row).  S is built
    on-device by the vector engine from packed val/rr arrays
    (S = (iota == rr) * val with 0-stride broadcast APs), not DMAed.
  - rrelu + down-cast is a single Prelu activation; layer-1 windows land in
    a transposed bf16 h1T tile that directly feeds the layer-2 table build
    (interleaved with layer-1's spmm); layer-2 windows land in a transposed
    fp16 out tile, written back per segment.
  - Host packs rows into windows (LPT on per-row A/B in-degree) so nearly
    every (window, group) hits its chunk budget exactly; the shared SPMD
    schedule is the per-window max over cores.
"""

import sys
import numpy as np

for _p in ("/opt/trn_rl_repo",):
    if _p not in sys.path:
        sys.path.insert(0, _p)

from ml_dtypes import bfloat16 as np_bf16

T, N, E, F = 4, 50000, 800000, 128
NC = 8
NPC = N // NC            # 6250 nodes per core
RTP = 6272               # padded rows per core (49 tiles of 128)
NT = RTP // 128          # 49 row tiles per core
WROWS = 64               # scatter window rows
NW = RTP // WROWS        # 98 windows per core
ACORES = 5               # table rows of cores [0,5) are group A
SPLIT = ACORES * RTP     # 31360 < 32768: both groups' indices fit int16
SLOPE = 11.0 / 48.0      # torch RReLU eval negative slope
SEGP = 2                 # row tiles per gather segment
TGT_A = 640              # per-window group-A edge target (5 chunks)
TGT_B = 384              # per-window group-B edge target (3 chunks)

SIM1 = False  # single-core, no-collective variant for TimelineSim
REPS = 1


def _evolve(W0, gW, gU, gb, steps=T):
    def sig(x):
        return 1.0 / (1.0 + np.exp(-x))

    Q = W0.astype(np.float64)
    gW = gW.astype(np.float64)
    gU = gU.astype(np.float64)
    gb = gb.astype(np.float64)
    for _ in range(steps):
        z = sig(gW[0] @ Q + gU[0] @ Q + gb[0])
        r = sig(gW[1] @ Q + gU[1] @ Q + gb[1])
        h = np.tanh(gW[2] @ Q + gU[2] @ (r * Q) + gb[2])
        Q = (1.0 - z) * Q + z * h
    return Q.astype(np.float32)


def _pack_windows(a, b, capA, capB, rng):
    """Assign rows (with group in-degrees a, b) of one shard to NW windows of
    64 slots, keeping window sums <= (capA[w], capB[w]).  Snake-deal by
    degree, then pairwise swap-repair of violations.  Returns positions."""
    n = len(a)
    order = np.argsort(-(a * 3 + b * 5), kind="stable")
    wins = np.empty(n, np.int64)
    rnds = np.arange(n) // NW
    js = np.arange(n) % NW
    wins[order] = np.where(rnds % 2 == 0, js, NW - 1 - js)

    def sums():
        A = np.bincount(wins, weights=a, minlength=NW).astype(np.int64)
        B = np.bincount(wins, weights=b, minlength=NW).astype(np.int64)
        return A, B

    A, B = sums()
    members = [list(np.nonzero(wins == w)[0]) for w in range(NW)]
    al = a.tolist()
    bl = b.tolist()
    capAl, capBl = capA.tolist(), capB.tolist()
    stuck = np.zeros(NW, bool)
    resets = 0
    for _it in range(6000):
        vA = np.maximum(A - capA, 0)
        vB = np.maximum(B - capB, 0)
        v = vA + vB
        va = v.copy()
        va[stuck] = 0
        if va.max() == 0:
            if v.max() == 0 or stuck.all() or resets >= 2:
                break
            stuck[:] = False
            resets += 1
            continue
        w = int(np.argmax(va))
        overA = bool(vA[w] > 0)
        overB = bool(vB[w] > 0)
        rw = members[w]
        sc_w = sorted(rw, key=lambda r: -(al[r] * overA + bl[r] * overB))[:10]
        roomA = capA - A
        roomB = capB - B
        cand_w2 = np.argpartition(-(roomA + roomB), 10)[:10]
        cand_w2 = cand_w2[np.argsort(-(roomA + roomB)[cand_w2])]
        done = False
        for r in sc_w:
            ar, br = al[r], bl[r]
            for w2 in cand_w2:
                if w2 == w:
                    continue
                w2 = int(w2)
                r2i = sorted(
                    members[w2],
                    key=lambda x: al[x] * overA + bl[x] * overB,
                )[:10]
                vold = int(v[w] + v[w2])
                for r2 in r2i:
                    a2, b2 = al[r2], bl[r2]
                    nA_w, nB_w = A[w] - ar + a2, B[w] - br + b2
                    nA_2, nB_2 = A[w2] + ar - a2, B[w2] + br - b2
                    new = (max(nA_w - capAl[w], 0) + max(nB_w - capBl[w], 0)
                           + max(nA_2 - capAl[w2], 0) + max(nB_2 - capBl[w2], 0))
                    if new < vold:
                        wins[r], wins[r2] = w2, w
                        members[w].remove(r)
                        members[w2].remove(r2)
                        members[w].append(r2)
                        members[w2].append(r)
                        A[w], B[w] = nA_w, nB_w
                        A[w2], B[w2] = nA_2, nB_2
                        done = True
                        break
                if done:
                    break
            if done:
                break
        if not done:
            stuck[w] = True
    pos = np.empty(n, np.int64)
    for w in range(NW):
        rows = np.nonzero(wins == w)[0]
        pos[rows] = w * WROWS + np.arange(len(rows))
    return pos


def _prep_edges(row, col, val):
    """Host-side schedule. Returns (sched, per-core input arrays)."""
    # ---- window packing -> within-shard positions
    gcol = (col // NPC) >= ACORES
    a_deg = np.bincount(row[~gcol], minlength=N)
    b_deg = np.bincount(row[gcol], minlength=N)
    # shared overflow-window profile: last KA/KB windows get one extra chunk
    a_tot = a_deg.reshape(NC, NPC).sum(axis=1)
    b_tot = b_deg.reshape(NC, NPC).sum(axis=1)
    KA = max(0, -(-(int(a_tot.max()) + 256 - NW * TGT_A) // 128))
    KB = max(0, -(-(int(b_tot.max()) + 256 - NW * TGT_B) // 128))
    capA = np.full(NW, TGT_A, np.int64)
    capA[NW - KA :] = TGT_A + 128
    capB = np.full(NW, TGT_B, np.int64)
    capB[NW - KB :] = TGT_B + 128
    pos = np.empty(N, np.int64)
    rng = np.random.default_rng(0)
    for i in range(NC):
        lo, hi = i * NPC, (i + 1) * NPC
        pos[lo:hi] = _pack_windows(a_deg[lo:hi], b_deg[lo:hi], capA, capB, rng)

    corei = row // NPC
    rl = pos[row]                       # scatter position within shard
    win = rl // WROWS
    rr = rl % WROWS
    tcol = (col // NPC) * RTP + pos[col]  # table row
    grp = (tcol >= SPLIT).astype(np.int64)

    # ---- merge exact duplicate (row, col) edges (S can only route a slot
    # to one destination row, so merging is valid only for identical rows)
    key = row * np.int64(N) + col
    order = np.argsort(key, kind="stable")
    key_s = key[order]
    uniq = np.empty(len(key_s), bool)
    uniq[0] = True
    uniq[1:] = key_s[1:] != key_s[:-1]
    seg_id = np.cumsum(uniq) - 1
    val_m = np.bincount(seg_id, weights=val[order].astype(np.float64))
    first = order[uniq]
    corei, win, rr, tcol, grp = (
        corei[first], win[first], rr[first], tcol[first], grp[first])
    val_m = val_m.astype(np.float32)

    # ---- shared chunk schedule: per (grp, win) max over cores
    counts = np.zeros((NC, 2, NW), np.int64)
    np.add.at(counts, (corei, grp, win), 1)
    CC = -(-counts // 128)
    CC = CC.max(axis=0)                 # [2, NW]
    CC[0] = np.maximum(CC[0], 1)        # every window needs >= 1 chunk
    baseA = np.zeros(NW + 1, np.int64)
    baseA[1:] = np.cumsum(CC[0])
    baseB = np.zeros(NW + 1, np.int64)
    baseB[1:] = np.cumsum(CC[1])
    NCHA, NCHB = int(baseA[-1]), int(baseB[-1])
    NCH = NCHA + NCHB
    NA, NB = NCHA * 128, NCHB * 128
    # unified S chunk ids, window-major (A then B within each window) so the
    # DVE S-build completes chunks in the order the spmm consumes them
    offW = np.zeros(NW + 1, np.int64)
    offW[1:] = np.cumsum(CC[0] + CC[1])

    idxa = np.zeros((NC, 128, NA // 16), np.int16)
    idxb = np.zeros((NC, 128, NB // 16), np.int16)
    valp = np.zeros((NC, 128, NCH), np.float16)
    rrp = np.full((NC, 128, NCH), 127.0, np.float16)

    for i in range(NC):
        for g, (base, idxg, idxoff) in enumerate(
            ((baseA, idxa, 0), (baseB, idxb, SPLIT))
        ):
            m = (corei == i) & (grp == g)
            ew, err = win[m], rr[m]
            etc = (tcol[m] - idxoff).astype(np.int16)
            ev = val_m[m]
            o = np.argsort(ew, kind="stable")
            ew, err, etc, ev = ew[o], err[o], etc[o], ev[o]
            winstart = np.searchsorted(ew, np.arange(NW))
            slot = base[ew] * 128 + (np.arange(ew.size) - winstart[ew])
            assert (slot < base[ew + 1] * 128).all()
            flat = np.zeros(base[-1] * 128, np.int16)
            flat[slot] = etc
            idxg[i][:16] = flat.reshape(-1, 16).T
            idxg[i] = np.tile(idxg[i][:16], (8, 1))
            p = slot % 128
            # unified chunk id: window-major
            gch = slot // 128                    # group-major chunk id
            loc = gch - base[ew]                 # chunk within window
            ch = offW[ew] + g * CC[0][ew] + loc
            valp[i, p, ch] = ev.astype(np.float16)
            rrp[i, p, ch] = err.astype(np.float16)

    sched = dict(
        CC=CC, baseA=baseA, baseB=baseB, NCHA=NCHA, NCHB=NCHB, offW=offW
    )
    return sched, pos, idxa, idxb, valp, rrp


def _build_program(sched):
    import concourse.bass as bass
    import concourse.tile as tile
    from concourse import bacc, mybir
    from contextlib import ExitStack

    F32, F16, BF16, I16 = (
        mybir.dt.float32, mybir.dt.float16, mybir.dt.bfloat16, mybir.dt.int16)
    baseA, baseB = sched["baseA"], sched["baseB"]
    NCHA, NCHB = sched["NCHA"], sched["NCHB"]
    offW = sched["offW"]
    CCA = sched["CC"][0]
    NCH = NCHA + NCHB
    NA, NB = NCHA * 128, NCHB * 128

    nc = bacc.Bacc(
        "TRN2", target_bir_lowering=False, debug=False,
        num_devices=(1 if SIM1 else NC),
    )
    xst_d = nc.dram_tensor("xst", [F, RTP], BF16, kind="ExternalInput")
    w1_d = nc.dram_tensor("w1", [F, F], BF16, kind="ExternalInput")
    w2_d = nc.dram_tensor("w2", [F, F], BF16, kind="ExternalInput")
    iota_d = nc.dram_tensor("iota", [128, WROWS], F16, kind="ExternalInput")
    idxa_d = nc.dram_tensor("idxa", [128, NA // 16], I16, kind="ExternalInput")
    idxb_d = nc.dram_tensor("idxb", [128, NB // 16], I16, kind="ExternalInput")
    valp_d = nc.dram_tensor("valp", [128, NCH], F16, kind="ExternalInput")
    rrp_d = nc.dram_tensor("rrp", [128, NCH], F16, kind="ExternalInput")
    out_d = nc.dram_tensor("out", [F, RTP], F16, kind="ExternalOutput")

    # gather segments: SEGP row tiles each
    WQ = 128 // WROWS
    segs = []
    for p0 in range(0, NT, SEGP):
        p1 = min(p0 + SEGP, NT)
        segs.append((p0, p1, p0 * WQ, p1 * WQ))
    max_cha = max(int(baseA[w1] - baseA[w0]) for _, _, w0, w1 in segs)
    max_chb = max(int(baseB[w1] - baseB[w0]) for _, _, w0, w1 in segs)

    with tile.TileContext(nc) as tc, ExitStack() as ctx:
        const = ctx.enter_context(tc.tile_pool(name="const", bufs=1))
        big = ctx.enter_context(tc.tile_pool(name="big", bufs=1))
        tps = ctx.enter_context(tc.tile_pool(name="tps", bufs=2, space="PSUM"))
        tsh = ctx.enter_context(tc.tile_pool(name="tsh", bufs=14))
        accp = ctx.enter_context(tc.tile_pool(name="accp", bufs=4, space="PSUM"))
        msgp = ctx.enter_context(tc.tile_pool(name="msgp", bufs=3))
        h1p = ctx.enter_context(tc.tile_pool(name="h1p", bufs=3))
        dram = ctx.enter_context(tc.tile_pool(name="dram", bufs=1, space="DRAM"))

        # table-build / allgather chunks (tile ranges), segment-aligned,
        # small tail chunk so the layer transition drains fast
        CHB = [0, 10, 20, 28, 36, 44, NT]
        NCHK = len(CHB) - 1

        # --- inputs with no deps first: fill DMA idle during table build
        w1_sb = const.tile([F, F], BF16)
        nc.sync.dma_start(w1_sb[:], w1_d[:, :])
        w2_sb = const.tile([F, F], BF16)
        nc.sync.dma_start(w2_sb[:], w2_d[:, :])
        xst_c = []
        for g in range(NCHK):
            t0, t1 = CHB[g], CHB[g + 1]
            xt = big.tile([F, (t1 - t0) * 128], BF16, name=f"xst{g}")
            nc.sync.dma_start(xt[:], xst_d[:, t0 * 128 : t1 * 128])
            xst_c.append(xt)
        iota_sb = const.tile([128, WROWS], F16)
        nc.sync.dma_start(iota_sb[:], iota_d[:, :])
        idxa_sb = big.tile([128, NA // 16], I16)
        nc.sync.dma_start(idxa_sb[:], idxa_d[:, :])
        idxb_sb = big.tile([128, NB // 16], I16)
        nc.sync.dma_start(idxb_sb[:], idxb_d[:, :])
        valp_sb = big.tile([128, NCH], F16)
        nc.sync.dma_start(valp_sb[:], valp_d[:, :])
        rrp_sb = big.tile([128, NCH], F16)
        nc.sync.dma_start(rrp_sb[:], rrp_d[:, :])

        s_sb = big.tile([128, NCH * WROWS], F16)

        def build_s():
            # S[p, cid*64 + j] = (iota[j] == rr[p,cid]) * val[p,cid], on DVE
            SLAB = 128
            for c0 in range(0, NCH, SLAB):
                c1 = min(c0 + SLAB, NCH)
                nch = c1 - c0
                s_slab = s_sb[:, c0 * WROWS : c1 * WROWS]
                s3 = s_slab.rearrange("p (c j) -> p c j", j=WROWS)
                iota_b = iota_sb[:, :].unsqueeze(1).broadcast_to([128, nch, WROWS])
                rr_b = rrp_sb[:, c0:c1].unsqueeze(2).broadcast_to([128, nch, WROWS])
                val_b = valp_sb[:, c0:c1].unsqueeze(2).broadcast_to([128, nch, WROWS])
                nc.vector.tensor_tensor(
                    out=s3, in0=iota_b, in1=rr_b, op=mybir.AluOpType.is_equal
                )
                nc.vector.tensor_tensor(
                    out=s3, in0=s3, in1=val_b, op=mybir.AluOpType.mult
                )

        def build_tiles(src_sb, src_t0, w_sb, shard, t0, t1, dma_eng=None):
            """table rows [t0*128, t1*128) = (src^T)[rows] @ w; one [128,128]
            matmul per tile, up to four tiles per copy/DMA.  dma_eng: queue
            for the shard writes (layer 1 uses Pool so the writes aren't
            stuck behind the input loads on the in-order SP queue)."""
            sh3 = shard.rearrange("(t q b) -> q t b", q=64, b=256)
            t = t0
            while t < t1:
                grp = min(4, t1 - t)
                ps = tps.tile([64, 1024], F32, tag="tp")
                for k in range(grp):
                    s0 = (t + k - src_t0) * 128
                    for par in range(2):
                        nc.tensor.matmul(
                            out=ps[:, k * 256 + par * 128 : k * 256 + (par + 1) * 128],
                            lhsT=src_sb[:, s0 + par : s0 + 128 : 2],
                            rhs=w_sb[:],
                            start=True, stop=True,
                        )
                sh = tsh.tile([64, 1024], F16, tag="sh")
                nc.scalar.activation(
                    sh[:, : grp * 256], ps[:, : grp * 256],
                    mybir.ActivationFunctionType.Copy,
                )
                sh_t = sh.rearrange("p (t b) -> p t b", b=256)
                (dma_eng or nc.sync).dma_start(
                    sh3[:, t : t + grp, :],
                    sh_t[:, :grp, :],
                )
                t += grp

        def all_gather(shard, table):
            if SIM1:
                for r in range(NC):
                    nc.sync.dma_start(
                        table[r * RTP * F : (r + 1) * RTP * F], shard[:]
                    )
            else:
                nc.gpsimd.collective_compute(
                    "AllGather",
                    mybir.AluOpType.bypass,
                    replica_groups=[list(range(NC))],
                    ins=[shard.opt()],
                    outs=[table.opt()],
                )

        # --- layer-1 table build (chunked for pipelining) + allgather
        _aspace = "Local" if SIM1 else "Shared"
        shard1 = dram.tile([RTP * F], F16, name="shard1")
        shard2 = dram.tile([RTP * F], F16, name="shard2")
        table1 = dram.tile([NC * RTP * F], F16, addr_space=_aspace, name="table1")
        table2 = dram.tile([NC * RTP * F], F16, addr_space=_aspace, name="table2")
        for g in range(NCHK):
            build_tiles(
                xst_c[g], CHB[g], w1_sb, shard1, CHB[g], CHB[g + 1],
            )
        build_s()
        all_gather(shard1, table1)

        def spmm(table, emit, interleave=None):
            tbl = table.rearrange("(r f) -> r f", f=F)
            for si, (p0, p1, w0, w1) in enumerate(segs):
                ca0, ca1 = int(baseA[w0]), int(baseA[w1])
                cb0, cb1 = int(baseB[w0]), int(baseB[w1])
                na, nb = (ca1 - ca0) * 128, (cb1 - cb0) * 128
                msga = msgp.tile([128, max_cha, 128], F16, tag="msga")
                msgb = msgp.tile([128, max_chb, 128], F16, tag="msgb")
                if na:
                    nc.gpsimd.dma_gather(
                        out_ap=msga[:, : ca1 - ca0, :],
                        in_ap=tbl[:SPLIT, :],
                        idxs_ap=idxa_sb[:, ca0 * 8 : ca1 * 8],
                        num_idxs=na,
                        num_idxs_reg=na,
                        elem_size=F,
                        single_packet=False,
                    )
                if nb:
                    nc.gpsimd.dma_gather(
                        out_ap=msgb[:, : cb1 - cb0, :],
                        in_ap=tbl[SPLIT:, :],
                        idxs_ap=idxb_sb[:, cb0 * 8 : cb1 * 8],
                        num_idxs=nb,
                        num_idxs_reg=nb,
                        elem_size=F,
                        single_packet=False,
                    )
                emt = emit(si)
                for w in range(w0, w1):
                    acc = accp.tile([128, WROWS], F32, tag="acc")
                    nw_ch = int(
                        baseA[w + 1] - baseA[w] + baseB[w + 1] - baseB[w]
                    )
                    k = 0
                    for gc in range(int(baseA[w]), int(baseA[w + 1])):
                        cid = int(offW[w]) + (gc - int(baseA[w]))
                        nc.tensor.matmul(
                            out=acc[:],
                            lhsT=msga[:, gc - ca0, :],
                            rhs=s_sb[:, cid * WROWS : (cid + 1) * WROWS],
                            start=(k == 0),
                            stop=(k == nw_ch - 1),
                        )
                        k += 1
                    for gc in range(int(baseB[w]), int(baseB[w + 1])):
                        cid = int(offW[w]) + int(CCA[w]) + (gc - int(baseB[w]))
                        nc.tensor.matmul(
                            out=acc[:],
                            lhsT=msgb[:, gc - cb0, :],
                            rhs=s_sb[:, cid * WROWS : (cid + 1) * WROWS],
                            start=(k == 0),
                            stop=(k == nw_ch - 1),
                        )
                        k += 1
                    emt(w - w0, acc)
                if interleave is not None:
                    interleave(si, p0, p1)

        # --- layer 1: spmm -> h1T (bf16, per-segment tiles) -> table2 build
        h1tiles = {}

        def emit1(si):
            h1t = h1p.tile([F, SEGP * 128], BF16, tag="h1t")
            h1tiles[si] = h1t

            def e(wloc, acc):
                nc.scalar.activation(
                    h1t[:, wloc * WROWS : (wloc + 1) * WROWS],
                    acc[:],
                    mybir.ActivationFunctionType.Prelu,
                    alpha=SLOPE,
                )
            return e

        def interleave1(si, p0, p1):
            build_tiles(h1tiles[si], p0, w2_sb, shard2, p0, p1)

        spmm(table1, emit1, interleave1)
        all_gather(shard2, table2)

        # --- layer 2: spmm -> outT fp16 -> DRAM per segment
        out_sb = big.tile([F, RTP], F16)

        def emit2(si):
            p0, p1, w0, w1 = segs[si]

            def e(wloc, acc):
                w = w0 + wloc
                nc.scalar.activation(
                    out_sb[:, w * WROWS : (w + 1) * WROWS],
                    acc[:],
                    mybir.ActivationFunctionType.Prelu,
                    alpha=SLOPE,
                )
            return e

        def interleave2(si, p0, p1):
            nc.sync.dma_start(
                out_d[:, p0 * 128 : p1 * 128],
                out_sb[:, p0 * 128 : p1 * 128],
            )

        spmm(table2, emit2, interleave2)

    nc.compile()
    return nc


def kernel(
    features,
    adj_row,
    adj_col,
    adj_val,
    W1,
    g1_W,
    g1_U,
    g1_b,
    W2,
    g2_W,
    g2_U,
    g2_b,
    _run_kwargs=None,
):
    from concourse.bass_utils import run_bass_kernel_spmd

    X = np.asarray(features[T - 1], dtype=np.float32)
    row = np.asarray(adj_row[T - 1], dtype=np.int64)
    col = np.asarray(adj_col[T - 1], dtype=np.int64)
    val = np.asarray(adj_val[T - 1], dtype=np.float32)

    W1f = _evolve(np.asarray(W1), np.asarray(g1_W), np.asarray(g1_U), np.asarray(g1_b))
    W2f = _evolve(np.asarray(W2), np.asarray(g2_W), np.asarray(g2_U), np.asarray(g2_b))

    sched, pos, idxa, idxb, valp, rrp = _prep_edges(row, col, val)
    nc = _build_program(sched)

    # xsT per core: [128, RTP] bf16, column pos[v] = X[v]
    xst = np.zeros((NC, F, RTP), np_bf16)
    for i in range(NC):
        lo, hi = i * NPC, (i + 1) * NPC
        xst[i][:, pos[lo:hi]] = X[lo:hi].T.astype(np_bf16)

    iota = np.broadcast_to(
        np.arange(WROWS, dtype=np.float16), (128, WROWS)
    ).copy()

    in_maps = [
        {
            "xst": xst[i],
            "w1": W1f.astype(np_bf16),
            "w2": W2f.astype(np_bf16),
            "iota": iota,
            "idxa": idxa[i],
            "idxb": idxb[i],
            "valp": valp[i],
            "rrp": rrp[i],
        }
        for i in range(NC)
    ]
    res = run_bass_kernel_spmd(
        nc, in_maps, core_ids=list(range(NC)), **(_run_kwargs or {})
    )
    out = np.empty((N, F), np.float32)
    for i in range(NC):
        lo, hi = i * NPC, (i + 1) * NPC
        arr = res.results[i]["out"].astype(np.float32)  # [F, RTP]
        out[lo:hi] = arr[:, pos[lo:hi]].T
    if _run_kwargs:
        kernel.last_results = res
    return out


# revision 45
# speedup vs baseline: 1.3676x; 1.0061x over previous
"""EvolveGCN (2-layer) Trainium2 Bass kernel, 8-way sharded.

Algebraic reduction: the mat-GRU evolving the GCN weights is data-independent
and only h2[T-1] is returned, so the whole model collapses to

    W1* = matGRU^4(W1);  W2* = matGRU^4(W2)      (tiny host math)
    h1  = rrelu(A3 @ (X3 @ W1*));  out = rrelu(A3 @ (h1 @ W2*))

Device schedule (per core, nodes range-partitioned by original id):
  - X arrives transposed bf16 [128F, RTP]; table build is a plain matmul
    lhsT=xsT slice (even/odd row split so the fp16 DRAM shard writes are
    512B-contiguous), PSUM->fp16 via Activation copy.
  - AllGather replicates the fp16 table [50176, 128] to every core.
  - SWDGE dma_gather pulls per-edge messages (one 256B descriptor per edge)
    group A (table rows < 5*RTP) / group B split so indices fit int16.
  - Segment-sum runs on the tensor engine: per 64-row window, PSUM
    accumulates accT[128F, 64rows] += msg_chunk.T @ S_chunk, where S
    [128 edge-slots, 64 rows] carries val at (slot, row).  S is built
    on-device by the vector engine from packed val/rr arrays
    (S = (iota == rr) * val with 0-stride broadcast APs), not DMAed.
  - rrelu + down-cast is a single Prelu activation; layer-1 windows land in
    a transposed bf16 h1T tile that directly feeds the layer-2 table build
    (interleaved with layer-1's spmm); layer-2 windows land in a transposed
    fp16 out tile, written back per segment.
  - Host packs rows into windows (LPT on per-row A/B in-degree) so nearly
    every (window, group) hits its chunk budget exactly; the shared SPMD
    schedule is the per-window max over cores.
"""

import sys
import numpy as np

for _p in ("/opt/trn_rl_repo",):
    if _p not in sys.path:
        sys.path.insert(0, _p)

from ml_dtypes import bfloat16 as np_bf16

T, N, E, F = 4, 50000, 800000, 128
NC = 8
NPC = N // NC            # 6250 nodes per core
RTP = 6272               # padded rows per core (49 tiles of 128)
NT = RTP // 128          # 49 row tiles per core
WROWS = 64               # scatter window rows
NW = RTP // WROWS        # 98 windows per core
ACORES = 5               # table rows of cores [0,5) are group A
SPLIT = ACORES * RTP     # 31360 < 32768: both groups' indices fit int16
SLOPE = 11.0 / 48.0      # torch RReLU eval negative slope
SEGP = 2                 # row tiles per gather segment
TGT_A = 640              # per-window group-A edge target (5 chunks)
TGT_B = 384              # per-window group-B edge target (3 chunks)

SIM1 = False  # single-core, no-collective variant for TimelineSim
REPS = 1


def _evolve(W0, gW, gU, gb, steps=T):
    def sig(x):
        return 1.0 / (1.0 + np.exp(-x))

    Q = W0.astype(np.float64)
    gW = gW.astype(np.float64)
    gU = gU.astype(np.float64)
    gb = gb.astype(np.float64)
    for _ in range(steps):
        z = sig(gW[0] @ Q + gU[0] @ Q + gb[0])
        r = sig(gW[1] @ Q + gU[1] @ Q + gb[1])
        h = np.tanh(gW[2] @ Q + gU[2] @ (r * Q) + gb[2])
        Q = (1.0 - z) * Q + z * h
    return Q.astype(np.float32)


def _pack_windows(a, b, capA, capB, rng, wa=3, wb=5):
    """Assign rows (with group in-degrees a, b) of one shard to NW windows of
    64 slots, keeping window sums <= (capA[w], capB[w]).  Snake-deal by
    degree, then pairwise swap-repair of violations.  Returns positions."""
    n = len(a)
    order = np.argsort(-(a * wa + b * wb), kind="stable")
    wins = np.empty(n, np.int64)
    rnds = np.arange(n) // NW
    js = np.arange(n) % NW
    wins[order] = np.where(rnds % 2 == 0, js, NW - 1 - js)

    def sums():
        A = np.bincount(wins, weights=a, minlength=NW).astype(np.int64)
        B = np.bincount(wins, weights=b, minlength=NW).astype(np.int64)
        return A, B

    A, B = sums()
    members = [list(np.nonzero(wins == w)[0]) for w in range(NW)]
    al = a.tolist()
    bl = b.tolist()
    capAl, capBl = capA.tolist(), capB.tolist()
    stuck = np.zeros(NW, bool)
    resets = 0
    for _it in range(20000):
        vA = np.maximum(A - capA, 0)
        vB = np.maximum(B - capB, 0)
        v = vA + vB
        va = v.copy()
        va[stuck] = 0
        if va.max() == 0:
            if v.max() == 0 or stuck.all() or resets >= 6:
                break
            stuck[:] = False
            resets += 1
            continue
        w = int(np.argmax(va))
        overA = bool(vA[w] > 0)
        overB = bool(vB[w] > 0)
        rw = members[w]
        sc_w = sorted(rw, key=lambda r: -(al[r] * overA + bl[r] * overB))[:10]
        roomA = capA - A
        roomB = capB - B
        cand_w2 = np.argpartition(-(roomA + roomB), 10)[:10]
        cand_w2 = cand_w2[np.argsort(-(roomA + roomB)[cand_w2])]
        done = False
        for r in sc_w:
            ar, br = al[r], bl[r]
            for w2 in cand_w2:
                if w2 == w:
                    continue
                w2 = int(w2)
                r2i = sorted(
                    members[w2],
                    key=lambda x: al[x] * overA + bl[x] * overB,
                )[:10]
                vold = int(v[w] + v[w2])
                for r2 in r2i:
                    a2, b2 = al[r2], bl[r2]
                    nA_w, nB_w = A[w] - ar + a2, B[w] - br + b2
                    nA_2, nB_2 = A[w2] + ar - a2, B[w2] + br - b2
                    new = (max(nA_w - capAl[w], 0) + max(nB_w - capBl[w], 0)
                           + max(nA_2 - capAl[w2], 0) + max(nB_2 - capBl[w2], 0))
                    if new < vold:
                        wins[r], wins[r2] = w2, w
                        members[w].remove(r)
                        members[w2].remove(r2)
                        members[w].append(r2)
                        members[w2].append(r)
                        A[w], B[w] = nA_w, nB_w
                        A[w2], B[w2] = nA_2, nB_2
                        done = True
                        break
                if done:
                    break
            if done:
                break
        if not done:
            stuck[w] = True
    pos = np.empty(n, np.int64)
    for w in range(NW):
        rows = np.nonzero(wins == w)[0]
        pos[rows] = w * WROWS + np.arange(len(rows))
    return pos


def _prep_edges(row, col, val):
    """Host-side schedule. Returns (sched, per-core input arrays)."""
    # ---- window packing -> within-shard positions
    gcol = (col // NPC) >= ACORES
    a_deg = np.bincount(row[~gcol], minlength=N)
    b_deg = np.bincount(row[gcol], minlength=N)
    # shared overflow-window profile: last KA/KB windows get one extra chunk
    a_tot = a_deg.reshape(NC, NPC).sum(axis=1)
    b_tot = b_deg.reshape(NC, NPC).sum(axis=1)
    KA = max(0, -(-(int(a_tot.max()) + 256 - NW * TGT_A) // 128))
    KB = max(0, -(-(int(b_tot.max()) + 256 - NW * TGT_B) // 128))
    capA = np.full(NW, TGT_A, np.int64)
    capA[NW - KA :] = TGT_A + 128
    capB = np.full(NW, TGT_B, np.int64)
    capB[NW - KB :] = TGT_B + 128
    pos = np.empty(N, np.int64)
    rng = np.random.default_rng(0)
    for i in range(NC):
        lo, hi = i * NPC, (i + 1) * NPC
        best = None
        for wa, wb in ((3, 5), (1, 1), (5, 3), (1, 3)):
            p = _pack_windows(
                a_deg[lo:hi], b_deg[lo:hi], capA, capB, rng, wa, wb
            )
            w = p // WROWS
            A = np.bincount(w, weights=a_deg[lo:hi], minlength=NW)
            B = np.bincount(w, weights=b_deg[lo:hi], minlength=NW)
            score = (
                np.maximum(-(-A.astype(np.int64) // 128) - capA // 128, 0).sum()
                + np.maximum(-(-B.astype(np.int64) // 128) - capB // 128, 0).sum()
            )
            if best is None or score < best[0]:
                best = (score, p)
            if score == 0:
                break
        pos[lo:hi] = best[1]

    corei = row // NPC
    rl = pos[row]                       # scatter position within shard
    win = rl // WROWS
    rr = rl % WROWS
    tcol = (col // NPC) * RTP + pos[col]  # table row
    grp = (tcol >= SPLIT).astype(np.int64)

    # ---- merge exact duplicate (row, col) edges (S can only route a slot
    # to one destination row, so merging is valid only for identical rows)
    key = row * np.int64(N) + col
    order = np.argsort(key, kind="stable")
    key_s = key[order]
    uniq = np.empty(len(key_s), bool)
    uniq[0] = True
    uniq[1:] = key_s[1:] != key_s[:-1]
    seg_id = np.cumsum(uniq) - 1
    val_m = np.bincount(seg_id, weights=val[order].astype(np.float64))
    first = order[uniq]
    corei, win, rr, tcol, grp = (
        corei[first], win[first], rr[first], tcol[first], grp[first])
    val_m = val_m.astype(np.float32)

    # ---- shared chunk schedule: per (grp, win) max over cores
    counts = np.zeros((NC, 2, NW), np.int64)
    np.add.at(counts, (corei, grp, win), 1)
    CC = -(-counts // 128)
    CC = CC.max(axis=0)                 # [2, NW]
    CC[0] = np.maximum(CC[0], 1)        # every window needs >= 1 chunk
    baseA = np.zeros(NW + 1, np.int64)
    baseA[1:] = np.cumsum(CC[0])
    baseB = np.zeros(NW + 1, np.int64)
    baseB[1:] = np.cumsum(CC[1])
    NCHA, NCHB = int(baseA[-1]), int(baseB[-1])
    NCH = NCHA + NCHB
    NA, NB = NCHA * 128, NCHB * 128
    # unified S chunk ids, window-major (A then B within each window) so the
    # DVE S-build completes chunks in the order the spmm consumes them
    offW = np.zeros(NW + 1, np.int64)
    offW[1:] = np.cumsum(CC[0] + CC[1])

    idxa = np.zeros((NC, 128, NA // 16), np.int16)
    idxb = np.zeros((NC, 128, NB // 16), np.int16)
    valp = np.zeros((NC, 128, NCH), np.float16)
    rrp = np.full((NC, 128, NCH), 127.0, np.float16)

    for i in range(NC):
        for g, (base, idxg, idxoff) in enumerate(
            ((baseA, idxa, 0), (baseB, idxb, SPLIT))
        ):
            m = (corei == i) & (grp == g)
            ew, err = win[m], rr[m]
            etc = (tcol[m] - idxoff).astype(np.int16)
            ev = val_m[m]
            o = np.argsort(ew, kind="stable")
            ew, err, etc, ev = ew[o], err[o], etc[o], ev[o]
            winstart = np.searchsorted(ew, np.arange(NW))
            slot = base[ew] * 128 + (np.arange(ew.size) - winstart[ew])
            assert (slot < base[ew + 1] * 128).all()
            flat = np.zeros(base[-1] * 128, np.int16)
            flat[slot] = etc
            idxg[i][:16] = flat.reshape(-1, 16).T
            idxg[i] = np.tile(idxg[i][:16], (8, 1))
            p = slot % 128
            # unified chunk id: window-major
            gch = slot // 128                    # group-major chunk id
            loc = gch - base[ew]                 # chunk within window
            ch = offW[ew] + g * CC[0][ew] + loc
            valp[i, p, ch] = ev.astype(np.float16)
            rrp[i, p, ch] = err.astype(np.float16)

    sched = dict(
        CC=CC, baseA=baseA, baseB=baseB, NCHA=NCHA, NCHB=NCHB, offW=offW
    )
    return sched, pos, idxa, idxb, valp, rrp


def _build_program(sched):
    import concourse.bass as bass
    import concourse.tile as tile
    from concourse import bacc, mybir
    from contextlib import ExitStack

    F32, F16, BF16, I16 = (
        mybir.dt.float32, mybir.dt.float16, mybir.dt.bfloat16, mybir.dt.int16)
    baseA, baseB = sched["baseA"], sched["baseB"]
    NCHA, NCHB = sched["NCHA"], sched["NCHB"]
    offW = sched["offW"]
    CCA = sched["CC"][0]
    NCH = NCHA + NCHB
    NA, NB = NCHA * 128, NCHB * 128

    nc = bacc.Bacc(
        "TRN2", target_bir_lowering=False, debug=False,
        num_devices=(1 if SIM1 else NC),
    )
    xst_d = nc.dram_tensor("xst", [F, RTP], BF16, kind="ExternalInput")
    w1_d = nc.dram_tensor("w1", [F, F], BF16, kind="ExternalInput")
    w2_d = nc.dram_tensor("w2", [F, F], BF16, kind="ExternalInput")
    iota_d = nc.dram_tensor("iota", [128, WROWS], F16, kind="ExternalInput")
    idxa_d = nc.dram_tensor("idxa", [128, NA // 16], I16, kind="ExternalInput")
    idxb_d = nc.dram_tensor("idxb", [128, NB // 16], I16, kind="ExternalInput")
    valp_d = nc.dram_tensor("valp", [128, NCH], F16, kind="ExternalInput")
    rrp_d = nc.dram_tensor("rrp", [128, NCH], F16, kind="ExternalInput")
    out_d = nc.dram_tensor("out", [F, RTP], F16, kind="ExternalOutput")

    # gather segments: SEGP row tiles each
    WQ = 128 // WROWS
    segs = []
    for p0 in range(0, NT, SEGP):
        p1 = min(p0 + SEGP, NT)
        segs.append((p0, p1, p0 * WQ, p1 * WQ))
    max_cha = max(int(baseA[w1] - baseA[w0]) for _, _, w0, w1 in segs)
    max_chb = max(int(baseB[w1] - baseB[w0]) for _, _, w0, w1 in segs)

    with tile.TileContext(nc) as tc, ExitStack() as ctx:
        const = ctx.enter_context(tc.tile_pool(name="const", bufs=1))
        big = ctx.enter_context(tc.tile_pool(name="big", bufs=1))
        tps = ctx.enter_context(tc.tile_pool(name="tps", bufs=2, space="PSUM"))
        tsh = ctx.enter_context(tc.tile_pool(name="tsh", bufs=14))
        accp = ctx.enter_context(tc.tile_pool(name="accp", bufs=4, space="PSUM"))
        msgp = ctx.enter_context(tc.tile_pool(name="msgp", bufs=3))
        h1p = ctx.enter_context(tc.tile_pool(name="h1p", bufs=3))
        dram = ctx.enter_context(tc.tile_pool(name="dram", bufs=1, space="DRAM"))

        # table-build / allgather chunks (tile ranges), segment-aligned,
        # small tail chunk so the layer transition drains fast
        CHB = [0, 10, 20, 28, 36, 44, NT]
        NCHK = len(CHB) - 1

        # --- inputs with no deps first: fill DMA idle during table build
        w1_sb = const.tile([F, F], BF16)
        nc.sync.dma_start(w1_sb[:], w1_d[:, :])
        w2_sb = const.tile([F, F], BF16)
        nc.sync.dma_start(w2_sb[:], w2_d[:, :])
        xst_c = []
        for g in range(NCHK):
            t0, t1 = CHB[g], CHB[g + 1]
            xt = big.tile([F, (t1 - t0) * 128], BF16, name=f"xst{g}")
            nc.sync.dma_start(xt[:], xst_d[:, t0 * 128 : t1 * 128])
            xst_c.append(xt)
        iota_sb = const.tile([128, WROWS], F16)
        nc.sync.dma_start(iota_sb[:], iota_d[:, :])
        idxa_sb = big.tile([128, NA // 16], I16)
        nc.sync.dma_start(idxa_sb[:], idxa_d[:, :])
        idxb_sb = big.tile([128, NB // 16], I16)
        nc.sync.dma_start(idxb_sb[:], idxb_d[:, :])
        valp_sb = big.tile([128, NCH], F16)
        nc.sync.dma_start(valp_sb[:], valp_d[:, :])
        rrp_sb = big.tile([128, NCH], F16)
        nc.sync.dma_start(rrp_sb[:], rrp_d[:, :])

        s_sb = big.tile([128, NCH * WROWS], F16)

        def build_s():
            # S[p, cid*64 + j] = (iota[j] == rr[p,cid]) * val[p,cid], on DVE
            SLAB = 128
            for c0 in range(0, NCH, SLAB):
                c1 = min(c0 + SLAB, NCH)
                nch = c1 - c0
                s_slab = s_sb[:, c0 * WROWS : c1 * WROWS]
                s3 = s_slab.rearrange("p (c j) -> p c j", j=WROWS)
                iota_b = iota_sb[:, :].unsqueeze(1).broadcast_to([128, nch, WROWS])
                rr_b = rrp_sb[:, c0:c1].unsqueeze(2).broadcast_to([128, nch, WROWS])
                val_b = valp_sb[:, c0:c1].unsqueeze(2).broadcast_to([128, nch, WROWS])
                nc.vector.tensor_tensor(
                    out=s3, in0=iota_b, in1=rr_b, op=mybir.AluOpType.is_equal
                )
                nc.vector.tensor_tensor(
                    out=s3, in0=s3, in1=val_b, op=mybir.AluOpType.mult
                )

        def build_tiles(src_sb, src_t0, w_sb, shard, t0, t1, dma_eng=None):
            """table rows [t0*128, t1*128) = (src^T)[rows] @ w; one [128,128]
            matmul per tile, up to four tiles per copy/DMA.  dma_eng: queue
            for the shard writes (layer 1 uses Pool so the writes aren't
            stuck behind the input loads on the in-order SP queue)."""
            sh3 = shard.rearrange("(t q b) -> q t b", q=64, b=256)
            t = t0
            while t < t1:
                grp = min(4, t1 - t)
                ps = tps.tile([64, 1024], F32, tag="tp")
                for k in range(grp):
                    s0 = (t + k - src_t0) * 128
                    for par in range(2):
                        nc.tensor.matmul(
                            out=ps[:, k * 256 + par * 128 : k * 256 + (par + 1) * 128],
                            lhsT=src_sb[:, s0 + par : s0 + 128 : 2],
                            rhs=w_sb[:],
                            start=True, stop=True,
                        )
                sh = tsh.tile([64, 1024], F16, tag="sh")
                nc.scalar.activation(
                    sh[:, : grp * 256], ps[:, : grp * 256],
                    mybir.ActivationFunctionType.Copy,
                )
                sh_t = sh.rearrange("p (t b) -> p t b", b=256)
                (dma_eng or nc.sync).dma_start(
                    sh3[:, t : t + grp, :],
                    sh_t[:, :grp, :],
                )
                t += grp

        def all_gather(shard, table):
            if SIM1:
                for r in range(NC):
                    nc.sync.dma_start(
                        table[r * RTP * F : (r + 1) * RTP * F], shard[:]
                    )
            else:
                nc.gpsimd.collective_compute(
                    "AllGather",
                    mybir.AluOpType.bypass,
                    replica_groups=[list(range(NC))],
                    ins=[shard.opt()],
                    outs=[table.opt()],
                )

        # --- layer-1 table build (chunked for pipelining) + allgather
        _aspace = "Local" if SIM1 else "Shared"
        shard1 = dram.tile([RTP * F], F16, name="shard1")
        shard2 = dram.tile([RTP * F], F16, name="shard2")
        table1 = dram.tile([NC * RTP * F], F16, addr_space=_aspace, name="table1")
        table2 = dram.tile([NC * RTP * F], F16, addr_space=_aspace, name="table2")
        for g in range(NCHK):
            build_tiles(
                xst_c[g], CHB[g], w1_sb, shard1, CHB[g], CHB[g + 1],
            )
        build_s()
        all_gather(shard1, table1)

        def spmm(table, emit, interleave=None):
            tbl = table.rearrange("(r f) -> r f", f=F)
            for si, (p0, p1, w0, w1) in enumerate(segs):
                ca0, ca1 = int(baseA[w0]), int(baseA[w1])
                cb0, cb1 = int(baseB[w0]), int(baseB[w1])
                na, nb = (ca1 - ca0) * 128, (cb1 - cb0) * 128
                msga = msgp.tile([128, max_cha, 128], F16, tag="msga")
                msgb = msgp.tile([128, max_chb, 128], F16, tag="msgb")
                if na:
                    nc.gpsimd.dma_gather(
                        out_ap=msga[:, : ca1 - ca0, :],
                        in_ap=tbl[:SPLIT, :],
                        idxs_ap=idxa_sb[:, ca0 * 8 : ca1 * 8],
                        num_idxs=na,
                        num_idxs_reg=na,
                        elem_size=F,
                        single_packet=False,
                    )
                if nb:
                    nc.gpsimd.dma_gather(
                        out_ap=msgb[:, : cb1 - cb0, :],
                        in_ap=tbl[SPLIT:, :],
                        idxs_ap=idxb_sb[:, cb0 * 8 : cb1 * 8],
                        num_idxs=nb,
                        num_idxs_reg=nb,
                        elem_size=F,
                        single_packet=False,
                    )
                emt = emit(si)
                for w in range(w0, w1):
                    acc = accp.tile([128, WROWS], F32, tag="acc")
                    nw_ch = int(
                        baseA[w + 1] - baseA[w] + baseB[w + 1] - baseB[w]
                    )
                    k = 0
                    for gc in range(int(baseA[w]), int(baseA[w + 1])):
                        cid = int(offW[w]) + (gc - int(baseA[w]))
                        nc.tensor.matmul(
                            out=acc[:],
                            lhsT=msga[:, gc - ca0, :],
                            rhs=s_sb[:, cid * WROWS : (cid + 1) * WROWS],
                            start=(k == 0),
                            stop=(k == nw_ch - 1),
                        )
                        k += 1
                    for gc in range(int(baseB[w]), int(baseB[w + 1])):
                        cid = int(offW[w]) + int(CCA[w]) + (gc - int(baseB[w]))
                        nc.tensor.matmul(
                            out=acc[:],
                            lhsT=msgb[:, gc - cb0, :],
                            rhs=s_sb[:, cid * WROWS : (cid + 1) * WROWS],
                            start=(k == 0),
                            stop=(k == nw_ch - 1),
                        )
                        k += 1
                    emt(w - w0, acc)
                if interleave is not None:
                    interleave(si, p0, p1)

        # --- layer 1: spmm -> h1T (bf16, per-segment tiles) -> table2 build
        h1tiles = {}

        def emit1(si):
            h1t = h1p.tile([F, SEGP * 128], BF16, tag="h1t")
            h1tiles[si] = h1t

            def e(wloc, acc):
                nc.scalar.activation(
                    h1t[:, wloc * WROWS : (wloc + 1) * WROWS],
                    acc[:],
                    mybir.ActivationFunctionType.Prelu,
                    alpha=SLOPE,
                )
            return e

        def interleave1(si, p0, p1):
            build_tiles(h1tiles[si], p0, w2_sb, shard2, p0, p1)

        spmm(table1, emit1, interleave1)
        all_gather(shard2, table2)

        # --- layer 2: spmm -> outT fp16 -> DRAM per segment
        out_sb = big.tile([F, RTP], F16)

        def emit2(si):
            p0, p1, w0, w1 = segs[si]

            def e(wloc, acc):
                w = w0 + wloc
                nc.scalar.activation(
                    out_sb[:, w * WROWS : (w + 1) * WROWS],
                    acc[:],
                    mybir.ActivationFunctionType.Prelu,
                    alpha=SLOPE,
                )
            return e

        def interleave2(si, p0, p1):
            nc.sync.dma_start(
                out_d[:, p0 * 128 : p1 * 128],
                out_sb[:, p0 * 128 : p1 * 128],
            )

        spmm(table2, emit2, interleave2)

    nc.compile()
    return nc


def kernel(
    features,
    adj_row,
    adj_col,
    adj_val,
    W1,
    g1_W,
    g1_U,
    g1_b,
    W2,
    g2_W,
    g2_U,
    g2_b,
    _run_kwargs=None,
):
    from concourse.bass_utils import run_bass_kernel_spmd

    X = np.asarray(features[T - 1], dtype=np.float32)
    row = np.asarray(adj_row[T - 1], dtype=np.int64)
    col = np.asarray(adj_col[T - 1], dtype=np.int64)
    val = np.asarray(adj_val[T - 1], dtype=np.float32)

    W1f = _evolve(np.asarray(W1), np.asarray(g1_W), np.asarray(g1_U), np.asarray(g1_b))
    W2f = _evolve(np.asarray(W2), np.asarray(g2_W), np.asarray(g2_U), np.asarray(g2_b))

    sched, pos, idxa, idxb, valp, rrp = _prep_edges(row, col, val)
    nc = _build_program(sched)

    # xsT per core: [128, RTP] bf16, column pos[v] = X[v]
    xst = np.zeros((NC, F, RTP), np_bf16)
    for i in range(NC):
        lo, hi = i * NPC, (i + 1) * NPC
        xst[i][:, pos[lo:hi]] = X[lo:hi].T.astype(np_bf16)

    iota = np.broadcast_to(
        np.arange(WROWS, dtype=np.float16), (128, WROWS)
    ).copy()

    in_maps = [
        {
            "xst": xst[i],
            "w1": W1f.astype(np_bf16),
            "w2": W2f.astype(np_bf16),
            "iota": iota,
            "idxa": idxa[i],
            "idxb": idxb[i],
            "valp": valp[i],
            "rrp": rrp[i],
        }
        for i in range(NC)
    ]
    res = run_bass_kernel_spmd(
        nc, in_maps, core_ids=list(range(NC)), **(_run_kwargs or {})
    )
    out = np.empty((N, F), np.float32)
    for i in range(NC):
        lo, hi = i * NPC, (i + 1) * NPC
        arr = res.results[i]["out"].astype(np.float32)  # [F, RTP]
        out[lo:hi] = arr[:, pos[lo:hi]].T
    if _run_kwargs:
        kernel.last_results = res
    return out
